# revision 8
# baseline (speedup 1.0000x reference)
"""Trainium2 Bass kernel for nn_Block_30313879175568 (dense transformer block).

Sharding: head-parallel attention (2 heads/core on 8 cores) + row-parallel
FFN/out-proj (1024 rows/core). Collectives: AllGather of rms-normed
activations (bf16, transposed layout), AllToAll of attention outputs
(heads -> rows). All matmul operands bf16 (fp32 PSUM accumulation); vector
math fp32.

Self-contained: imports only installed packages (concourse et al.) + numpy.
"""

import numpy as np
import ml_dtypes

import concourse.bass as bass  # noqa: F401
import concourse.mybir as mybir
import concourse.tile as tile
from concourse import bacc
from concourse.bass_utils import run_bass_kernel_spmd

BF16 = mybir.dt.bfloat16
F32 = mybir.dt.float32
AF = mybir.ActivationFunctionType
ALU = mybir.AluOpType

B, L, DIM, H, HID = 4, 2048, 1024, 16, 4096
HEAD_DIM = 64
NCORES = 8
R = B * L              # 8192 global rows
RC = R // NCORES       # 1024 rows per core
HPC = H // NCORES      # 2 heads per core
RMS_EPS = 1e-6
LN_EPS = 1e-5
VCOL = 2 * (HEAD_DIM + 1)   # 130: V cols per (batch,ktile) block incl ones

_PROGRAM_CACHE = {}
_LAST_IN_MAPS = None


# ----------------------------------------------------------------------------
# host-side helpers
# ----------------------------------------------------------------------------

def _bf16(a):
    return np.asarray(a, dtype=np.float32).astype(ml_dtypes.bfloat16)


def _rope_tables():
    half = HEAD_DIM // 2
    inv_freq = 10000.0 ** (-np.arange(0, half, dtype=np.float32) * 2.0 / HEAD_DIM)
    pos = np.arange(L, dtype=np.float32)
    theta = pos[:, None] * inv_freq[None, :]          # [L, 32]
    return (np.cos(theta).T.astype(np.float32).copy(),
            np.sin(theta).T.astype(np.float32).copy())  # [32, L]


def _classify_mask(mask):
    """Split mask^T [k, q] into (16 ktile x 4 qblock) blocks.

    Returns (actions, pmask_np): actions[(kt, qb)] is 'skip' | 'noop' |
    ('mul', idx); pmask_np is [NU, 128, 512] bf16 of exp(mask^T block).
    """
    maskT = np.asarray(mask, dtype=np.float32).T
    actions = {}
    uniq = {}
    tiles = []
    for qb in range(4):
        for kt in range(16):
            blk = maskT[128 * kt:128 * (kt + 1), 512 * qb:512 * (qb + 1)]
            if np.all(blk <= -30.0):
                actions[(kt, qb)] = "skip"
            elif np.all(blk == 0.0):
                actions[(kt, qb)] = "noop"
            else:
                pm = _bf16(np.exp(blk.astype(np.float64)))
                key = pm.tobytes()
                if key not in uniq:
                    uniq[key] = len(tiles)
                    tiles.append(pm)
                actions[(kt, qb)] = ("mul", uniq[key])
    if not tiles:
        tiles = [np.zeros((128, 512), dtype=ml_dtypes.bfloat16)]
    pmask_np = np.stack(tiles, axis=0)
    return actions, pmask_np


def _numpy_fallback(x, mask, attn_scale, wqkv_w, wqkv_b, out_w, out_b,
                    ffn_scale, lin1_w, lin1_b, ln_g, ln_b, lin2_w, lin2_b):
    """Correct (slow) host fallback for configurations the device program
    doesn't support (nonzero biases / fully-masked rows)."""
    from scipy.special import erf

    def rms(t, scale):
        return t / np.sqrt(np.mean(t * t, axis=-1, keepdims=True) + RMS_EPS) * scale

    x = np.asarray(x, np.float64)
    xn = rms(x, attn_scale)
    qkv = xn @ np.asarray(wqkv_w, np.float64) + wqkv_b
    q, k, v = np.split(qkv, 3, axis=-1)
    th = lambda t: t.reshape(B, L, H, HEAD_DIM).transpose(0, 2, 1, 3)
    q, k, v = th(q), th(k), th(v)

    half = HEAD_DIM // 2
    inv_freq = 10000.0 ** (-np.arange(0, half) * 2.0 / HEAD_DIM)
    theta = np.arange(L)[:, None] * inv_freq[None, :]
    cos, sin = np.cos(theta), np.sin(theta)

    def rope(t):
        x1, x2 = t[..., :half], t[..., half:]
        return np.concatenate([x1 * cos - x2 * sin, x1 * sin + x2 * cos], axis=-1)

    q, k = rope(q), rope(k)
    s = np.einsum("bhqd,bhkd->bhqk", q / np.sqrt(HEAD_DIM), k) + np.asarray(mask, np.float64)
    s = s - s.max(axis=-1, keepdims=True)
    p = np.exp(s)
    p /= p.sum(axis=-1, keepdims=True)
    o = np.einsum("bhqk,bhkd->bhqd", p, v)
    o = o.transpose(0, 2, 1, 3).reshape(B, L, DIM)
    h = x + o @ np.asarray(out_w, np.float64) + out_b
    f = rms(h, ffn_scale)
    f = f @ np.asarray(lin1_w, np.float64) + lin1_b
    f = 0.5 * f * (1.0 + erf(f / np.sqrt(2.0)))
    mu = f.mean(axis=-1, keepdims=True)
    var = f.var(axis=-1, keepdims=True)
    f = (f - mu) / np.sqrt(var + LN_EPS) * ln_g + ln_b
    out = h + f @ np.asarray(lin2_w, np.float64) + lin2_b
    return out.astype(np.float32)


# ----------------------------------------------------------------------------
# device program
# ----------------------------------------------------------------------------

def _rms_rstd(nc, stats, t, eps):
    """1/sqrt(mean(t^2, free) + eps) for a [128, D] f32 tile (D mult of 512)."""
    D = t.shape[1]
    g = D // 512
    st = stats.tile([128, g, 6], F32, tag="rmsst", name="rmsst")
    tv = t[:].rearrange("p (g f) -> p g f", g=g)
    for i in range(g):
        nc.vector.bn_stats(st[:, i, :], tv[:, i, :])
    mv = stats.tile([128, 2], F32, tag="rmsmv", name="rmsmv")
    nc.vector.bn_aggr(mv[:], st[:])
    ms = stats.tile([128, 1], F32, tag="rmsms", name="rmsms")
    nc.vector.tensor_scalar(ms[:], mv[:, 0:1], mv[:, 0:1], None, ALU.mult)
    nc.vector.tensor_add(ms[:], ms[:], mv[:, 1:2])
    std = stats.tile([128, 1], F32, tag="rmssd", name="rmssd")
    nc.scalar.activation(std[:], ms[:], AF.Sqrt, bias=eps, scale=1.0)
    rstd = stats.tile([128, 1], F32, tag="rmsrs", name="rmsrs")
    nc.vector.reciprocal(rstd[:], std[:])
    return rstd


def _register_const(nc, value, dtype=F32):
    t = nc.alloc_sbuf_tensor(f"const-{dtype.name}-{value}", [128, 1], dtype)
    nc.gpsimd.memset(t.ap(), value)
    nc.const_aps.aps[(dtype, value)] = t.ap()


def _build_program(actions, n_pmask):
    nc = bacc.Bacc("TRN2", target_bir_lowering=False, debug=False,
                   num_devices=NCORES)
    _register_const(nc, RMS_EPS)
    _register_const(nc, LN_EPS)
    nc.all_engine_barrier()

    x_in = nc.dram_tensor("x_own", [RC, DIM], F32, kind="ExternalInput")
    wqkv_in = nc.dram_tensor("wqkv_sl", [DIM, 3 * 128], BF16, kind="ExternalInput")
    outw_in = nc.dram_tensor("out_w", [DIM, DIM], BF16, kind="ExternalInput")
    l1w_in = nc.dram_tensor("lin1_w", [DIM, HID], BF16, kind="ExternalInput")
    l2w_in = nc.dram_tensor("lin2_w", [HID, DIM], BF16, kind="ExternalInput")
    cos_in = nc.dram_tensor("cosT", [32, L], F32, kind="ExternalInput")
    sin_in = nc.dram_tensor("sinT", [32, L], F32, kind="ExternalInput")
    pm_in = nc.dram_tensor("pmask", [n_pmask, 128, 512], BF16, kind="ExternalInput")
    eye_in = nc.dram_tensor("eye", [128, 128], BF16, kind="ExternalInput")
    y_out = nc.dram_tensor("y_own", [RC, DIM], F32, kind="ExternalOutput")

    with tile.TileContext(nc) as tc:
        _emit(nc, tc, x_in, wqkv_in, outw_in, l1w_in, l2w_in, cos_in, sin_in,
              pm_in, eye_in, y_out, actions, n_pmask)

    nc.compile()
    return nc


def _emit(nc, tc, x_in, wqkv_in, outw_in, l1w_in, l2w_in, cos_in, sin_in,
          pm_in, eye_in, y_out, actions, n_pmask):
    with (
        tc.tile_pool(name="dram", bufs=1, space="DRAM") as dram,
        tc.tile_pool(name="base", bufs=1) as base,
        tc.tile_pool(name="work", bufs=4) as work,
        tc.tile_pool(name="stats", bufs=4) as stats,
    ):
        eye = base.tile([128, 128], BF16)
        nc.sync.dma_start(eye[:], eye_in.ap())

        ag_in = dram.tile([DIM, RC], BF16)
        ag_out = dram.tile([NCORES * DIM, RC], BF16, addr_space="Shared")
        a2a_in = dram.tile([DIM, RC], BF16)
        a2a_out = dram.tile([DIM, RC], BF16)

        with tc.tile_pool(name="xp", bufs=1) as xp:
            x_t = [xp.tile([128, DIM], F32, tag=f"x{rt}", name=f"x{rt}")
                   for rt in range(8)]

            # ---------------- phase 1: load x, rms-norm, transpose -> ag_in
            with (
                tc.tile_pool(name="ps1", bufs=2, space="PSUM") as ps1,
                tc.tile_pool(name="p1", bufs=3) as p1,
            ):
                for rt in range(8):
                    nc.sync.dma_start(x_t[rt][:], x_in.ap()[128 * rt:128 * (rt + 1), :])
                    rstd = _rms_rstd(nc, stats, x_t[rt], RMS_EPS)
                    xn = p1.tile([128, DIM], BF16, tag="xn", name="xn")
                    nc.scalar.activation(xn[:], x_t[rt][:], AF.Copy, bias=0.0,
                                         scale=rstd[:])
                    for fc in range(8):
                        ps = ps1.tile([128, 128], BF16, tag="tr", name="tr")
                        nc.tensor.transpose(ps[:], xn[:, 128 * fc:128 * (fc + 1)], eye[:])
                        ev = p1.tile([128, 128], BF16, tag="ev", name="ev")
                        nc.vector.tensor_copy(ev[:], ps[:])
                        nc.sync.dma_start(
                            ag_in[128 * fc:128 * (fc + 1), 128 * rt:128 * (rt + 1)],
                            ev[:])

            # ---------------- phase 2: AllGather xn^T (bf16)
            nc.gpsimd.collective_compute(
                "AllGather", ALU.bypass,
                replica_groups=[list(range(NCORES))],
                ins=[ag_in[:].opt()], outs=[ag_out[:].opt()])

            with (
                tc.tile_pool(name="attn", bufs=1) as attn,
                tc.tile_pool(name="cst", bufs=1) as cst,
            ):
                cosT = cst.tile([32, L], F32)
                sinT = cst.tile([32, L], F32)
                nc.sync.dma_start(cosT[:], cos_in.ap())
                nc.sync.dma_start(sinT[:], sin_in.ap())
                pm_t = []
                for i in range(n_pmask):
                    t = cst.tile([128, 512], BF16, tag=f"pm{i}", name=f"pm{i}")
                    nc.sync.dma_start(t[:], pm_in.ap()[i, :, :])
                    pm_t.append(t)
                w_t = []
                for kc in range(8):
                    t = cst.tile([128, 3 * 128], BF16, tag=f"w{kc}", name=f"w{kc}")
                    nc.sync.dma_start(t[:], wqkv_in.ap()[128 * kc:128 * (kc + 1), :])
                    w_t.append(t)

                QT = attn.tile([128, R], BF16)     # [2heads x 64dim, global row]
                KT = attn.tile([128, R], BF16)
                VT = attn.tile([128, R], BF16)
                V_sb = attn.tile([128, 64 * VCOL], BF16)
                nc.vector.memset(V_sb[:], 1.0)

                # ------------ phase 3: QKV + RoPE
                with (
                    tc.tile_pool(name="ps3", bufs=2, space="PSUM") as ps3,
                    tc.tile_pool(name="p3", bufs=3) as p3,
                ):
                    for blk in range(16):
                        cb, lo = blk // 2, 512 * (blk % 2)
                        pos0 = 512 * (blk % 4)
                        g0 = 512 * blk
                        xnt = []
                        for kc in range(8):
                            t = p3.tile([128, 512], BF16, tag="xnt", name="xnt",
                                        bufs=12)
                            nc.sync.dma_start(
                                t[:],
                                ag_out[DIM * cb + 128 * kc:DIM * cb + 128 * (kc + 1),
                                       lo:lo + 512])
                            xnt.append(t)
                        psQ = ps3.tile([128, 512], F32, tag="q", name="psq")
                        psK = ps3.tile([128, 512], F32, tag="k", name="psk")
                        psV = ps3.tile([128, 512], F32, tag="v", name="psv")
                        for kc in range(8):
                            st, sp = kc == 0, kc == 7
                            nc.tensor.matmul(psQ[:], w_t[kc][:, 0:128], xnt[kc][:],
                                             start=st, stop=sp)
                            nc.tensor.matmul(psK[:], w_t[kc][:, 128:256], xnt[kc][:],
                                             start=st, stop=sp)
                            nc.tensor.matmul(psV[:], w_t[kc][:, 256:384], xnt[kc][:],
                                             start=st, stop=sp)
                        cs = cosT[:, pos0:pos0 + 512]
                        sn = sinT[:, pos0:pos0 + 512]
                        for ps, dst in ((psQ, QT), (psK, KT)):
                            for h0 in (0, 64):
                                x1 = ps[h0:h0 + 32, :]
                                x2 = ps[h0 + 32:h0 + 64, :]
                                t1 = work.tile([32, 512], F32, tag="r1", name="r1")
                                t2 = work.tile([32, 512], F32, tag="r2", name="r2")
                                nc.vector.tensor_mul(t1[:], x1, cs)
                                nc.vector.tensor_mul(t2[:], x2, sn)
                                nc.vector.tensor_sub(dst[h0:h0 + 32, g0:g0 + 512],
                                                     t1[:], t2[:])
                                nc.vector.tensor_mul(t1[:], x1, sn)
                                nc.vector.tensor_mul(t2[:], x2, cs)
                                nc.vector.tensor_add(dst[h0 + 32:h0 + 64, g0:g0 + 512],
                                                     t1[:], t2[:])
                        nc.vector.tensor_copy(VT[:, g0:g0 + 512], psV[:])

                    # ------------ phase 4: V^T -> row-major V blocks (+ones)
                    for bt in range(64):
                        pst = ps3.tile([128, 128], BF16, tag="tr", name="tr")
                        nc.tensor.transpose(pst[:], VT[:, 128 * bt:128 * (bt + 1)],
                                            eye[:])
                        nc.vector.tensor_copy(V_sb[:, VCOL * bt:VCOL * bt + 64],
                                              pst[:, 0:64])
                        nc.vector.tensor_copy(V_sb[:, VCOL * bt + 65:VCOL * bt + 129],
                                              pst[:, 64:128])

                # ------------ phase 5: attention per (batch, head, qblock)
                with (
                    tc.tile_pool(name="ps5s", bufs=4, space="PSUM") as ps5s,
                    tc.tile_pool(name="ps5o", bufs=2, space="PSUM") as ps5o,
                    tc.tile_pool(name="p5", bufs=3) as p5,
                ):
                    for b in range(B):
                        for h in range(HPC):
                            hr = 64 * h
                            for qb in range(4):
                                q0 = 2048 * b + 512 * qb
                                act = [(kt, actions[(kt, qb)]) for kt in range(16)
                                       if actions[(kt, qb)] != "skip"]
                                psO = ps5o.tile([65, 512], F32, tag="o", name="pso")
                                pts = []
                                for kt, a in act:
                                    k0 = 2048 * b + 128 * kt
                                    psS = ps5s.tile([128, 512], F32, tag="s",
                                                    name="pss")
                                    nc.tensor.matmul(
                                        psS[:], KT[hr:hr + 64, k0:k0 + 128],
                                        QT[hr:hr + 64, q0:q0 + 512],
                                        start=True, stop=True)
                                    pt = p5.tile([128, 512], BF16, tag="pt",
                                                 name="pt", bufs=18)
                                    nc.scalar.activation(pt[:], psS[:], AF.Exp,
                                                         bias=0.0, scale=0.125)
                                    if a != "noop":
                                        nc.vector.tensor_mul(pt[:], pt[:],
                                                             pm_t[a[1]][:])
                                    pts.append((kt, pt))
                                for i, (kt, pt) in enumerate(pts):
                                    bt = 16 * b + kt
                                    nc.tensor.matmul(
                                        psO[:],
                                        V_sb[:, VCOL * bt + 65 * h:
                                             VCOL * bt + 65 * h + 65],
                                        pt[:],
                                        start=(i == 0), stop=(i == len(pts) - 1))
                                rec = stats.tile([1, 512], F32, tag="rec",
                                                 name="rec")
                                nc.vector.reciprocal(rec[:], psO[64:65, :])
                                rb = p5.tile([64, 512], F32, tag="rb", name="rb")
                                nc.gpsimd.partition_broadcast(rb[:], rec[:])
                                ot = p5.tile([64, 512], BF16, tag="ot", name="ot")
                                nc.vector.tensor_mul(ot[:], psO[0:64, :], rb[:])
                                j = (2048 * b + 512 * qb) // RC
                                loc0 = 2048 * b + 512 * qb - RC * j
                                nc.sync.dma_start(
                                    a2a_in[128 * j + hr:128 * j + hr + 64,
                                           loc0:loc0 + 512],
                                    ot[:])

            # ---------------- phase 6: AllToAll o^T heads -> rows
            nc.gpsimd.collective_compute(
                "AllToAll", ALU.bypass,
                replica_groups=[list(range(NCORES))],
                ins=[a2a_in[:].opt()], outs=[a2a_out[:].opt()])

            # ---------------- phase 7: out-proj + residual -> h
            with tc.tile_pool(name="hp", bufs=1) as hp:
                h_t = [hp.tile([128, DIM], F32, tag=f"h{rt}", name=f"h{rt}")
                       for rt in range(8)]
                with (
                    tc.tile_pool(name="ps7", bufs=1, space="PSUM") as ps7,
                    tc.tile_pool(name="p7", bufs=1) as p7,
                    tc.tile_pool(name="p7w", bufs=6) as p7w,
                ):
                    oT = []
                    for kc in range(8):
                        t = p7.tile([128, RC], BF16, tag=f"ot{kc}", name=f"oT{kc}")
                        nc.sync.dma_start(t[:], a2a_out[128 * kc:128 * (kc + 1), :])
                        oT.append(t)
                    for nb in range(2):
                        pss = [ps7.tile([128, 512], F32, tag=f"mm{rt % 4}",
                                        name="psmm", bufs=2) for rt in range(8)]
                        for kc in range(8):
                            w = p7w.tile([128, 512], BF16, tag="ow", name="ow")
                            nc.sync.dma_start(
                                w[:], outw_in.ap()[128 * kc:128 * (kc + 1),
                                                   512 * nb:512 * (nb + 1)])
                            for rt in range(8):
                                nc.tensor.matmul(pss[rt][:],
                                                 oT[kc][:, 128 * rt:128 * (rt + 1)],
                                                 w[:], start=(kc == 0),
                                                 stop=(kc == 7))
                        for rt in range(8):
                            nc.vector.tensor_add(
                                h_t[rt][:, 512 * nb:512 * (nb + 1)], pss[rt][:],
                                x_t[rt][:, 512 * nb:512 * (nb + 1)])

                # ------------ phase 8: FFN (row-local), two halves of 512 rows
                with (
                    tc.tile_pool(name="ps8", bufs=1, space="PSUM") as ps8,
                    tc.tile_pool(name="ps8t", bufs=2, space="PSUM") as ps8t,
                    tc.tile_pool(name="p8", bufs=1) as p8,
                    tc.tile_pool(name="p8w", bufs=6) as p8w,
                    tc.tile_pool(name="p8s", bufs=3) as p8s,
                ):
                    for half in range(2):
                        # rms-norm h -> fn (bf16) -> transpose -> fnT
                        fnT = [p8.tile([128, 512], BF16, tag=f"fnT{fc}",
                                       name=f"fnT{fc}") for fc in range(8)]
                        for rt2 in range(4):
                            rt = 4 * half + rt2
                            rstd = _rms_rstd(nc, stats, h_t[rt], RMS_EPS)
                            fn = p8s.tile([128, DIM], BF16, tag="fn", name="fn")
                            nc.scalar.activation(fn[:], h_t[rt][:], AF.Copy,
                                                 bias=0.0, scale=rstd[:])
                            for fc in range(8):
                                ps = ps8t.tile([128, 128], BF16, tag="tr", name="tr")
                                nc.tensor.transpose(
                                    ps[:], fn[:, 128 * fc:128 * (fc + 1)], eye[:])
                                nc.vector.tensor_copy(
                                    fnT[fc][:, 128 * rt2:128 * (rt2 + 1)], ps[:])
                        # lin1 + GELU -> g [4 x 8 tiles of [128,512] bf16]
                        g_t = [[p8.tile([128, 512], BF16, tag=f"g{rt2}_{hb}",
                                        name=f"g{rt2}_{hb}")
                                for hb in range(8)] for rt2 in range(4)]
                        for hb in range(8):
                            pss = [ps8.tile([128, 512], F32, tag=f"mm{rt2}",
                                            name="psmm", bufs=1)
                                   for rt2 in range(4)]
                            for fc in range(8):
                                w = p8w.tile([128, 512], BF16, tag="l1w", name="l1w")
                                nc.sync.dma_start(
                                    w[:], l1w_in.ap()[128 * fc:128 * (fc + 1),
                                                      512 * hb:512 * (hb + 1)])
                                for rt2 in range(4):
                                    nc.tensor.matmul(
                                        pss[rt2][:],
                                        fnT[fc][:, 128 * rt2:128 * (rt2 + 1)],
                                        w[:], start=(fc == 0), stop=(fc == 7))
                            for rt2 in range(4):
                                nc.scalar.activation(g_t[rt2][hb][:], pss[rt2][:],
                                                     AF.Gelu)
                        # LayerNorm stats over hid (4096) per row
                        ab = []
                        for rt2 in range(4):
                            st = stats.tile([128, 8, 6], F32, tag="lnst",
                                            name="lnst")
                            for hb in range(8):
                                nc.vector.bn_stats(st[:, hb, :], g_t[rt2][hb][:])
                            mv = stats.tile([128, 2], F32, tag="lnmv", name="lnmv")
                            nc.vector.bn_aggr(mv[:], st[:])
                            std = stats.tile([128, 1], F32, tag="lnsd", name="lnsd")
                            nc.scalar.activation(std[:], mv[:, 1:2], AF.Sqrt,
                                                 bias=LN_EPS, scale=1.0)
                            rstd = stats.tile([128, 1], F32, tag="lnrs",
                                              name="lnrs")
                            nc.vector.reciprocal(rstd[:], std[:])
                            nmr = stats.tile([128, 1], F32, tag="lnnm", name="lnnm")
                            nc.vector.tensor_scalar(nmr[:], rstd[:], mv[:, 0:1],
                                                    -1.0, ALU.mult, ALU.mult)
                            ab.append((rstd, nmr))
                        # normalize + transpose -> gnT [32 tiles of [128,512]]
                        gnT = [p8.tile([128, 512], BF16, tag=f"gnT{hc}",
                                       name=f"gnT{hc}") for hc in range(32)]
                        for rt2 in range(4):
                            rstd, nmr = ab[rt2]
                            for hb in range(8):
                                gn = p8s.tile([128, 512], BF16, tag="gn", name="gn")
                                nc.vector.tensor_scalar(gn[:], g_t[rt2][hb][:],
                                                        rstd[:], nmr[:],
                                                        ALU.mult, ALU.add)
                                for j in range(4):
                                    ps = ps8t.tile([128, 128], BF16, tag="tr",
                                                   name="tr")
                                    nc.tensor.transpose(
                                        ps[:], gn[:, 128 * j:128 * (j + 1)], eye[:])
                                    nc.vector.tensor_copy(
                                        gnT[4 * hb + j][:, 128 * rt2:128 * (rt2 + 1)],
                                        ps[:])
                        # lin2 + residual -> y
                        for nb in range(2):
                            pss = [ps8.tile([128, 512], F32, tag=f"mm{rt2}",
                                            name="psmm", bufs=1)
                                   for rt2 in range(4)]
                            for hc in range(32):
                                w = p8w.tile([128, 512], BF16, tag="l2w", name="l2w")
                                nc.sync.dma_start(
                                    w[:], l2w_in.ap()[128 * hc:128 * (hc + 1),
                                                      512 * nb:512 * (nb + 1)])
                                for rt2 in range(4):
                                    nc.tensor.matmul(
                                        pss[rt2][:],
                                        gnT[hc][:, 128 * rt2:128 * (rt2 + 1)],
                                        w[:], start=(hc == 0), stop=(hc == 31))
                            for rt2 in range(4):
                                rt = 4 * half + rt2
                                yt = p8s.tile([128, 512], F32, tag="yt", name="yt")
                                nc.vector.tensor_add(
                                    yt[:], pss[rt2][:],
                                    h_t[rt][:, 512 * nb:512 * (nb + 1)])
                                nc.sync.dma_start(
                                    y_out.ap()[128 * rt:128 * (rt + 1),
                                               512 * nb:512 * (nb + 1)],
                                    yt[:])


# ----------------------------------------------------------------------------
# entry point
# ----------------------------------------------------------------------------

def kernel(x, mask, attn_scale, wqkv_w, wqkv_b, out_w, out_b,
           ffn_scale, lin1_w, lin1_b, ln_g, ln_b, lin2_w, lin2_b):
    x = np.asarray(x, np.float32)
    mask = np.asarray(mask, np.float32)

    lin2_b_eff = (np.asarray(lin2_b, np.float32)
                  + np.asarray(ln_b, np.float32) @ np.asarray(lin2_w, np.float32))
    if np.any(wqkv_b) or np.any(out_b) or np.any(lin1_b) or np.any(lin2_b_eff):
        return _numpy_fallback(x, mask, attn_scale, wqkv_w, wqkv_b, out_w, out_b,
                               ffn_scale, lin1_w, lin1_b, ln_g, ln_b, lin2_w,
                               lin2_b)

    actions, pmask_np = _classify_mask(mask)
    for qb in range(4):
        if all(actions[(kt, qb)] == "skip" for kt in range(16)):
            return _numpy_fallback(x, mask, attn_scale, wqkv_w, wqkv_b, out_w,
                                   out_b, ffn_scale, lin1_w, lin1_b, ln_g, ln_b,
                                   lin2_w, lin2_b)

    mask_sig = tuple(sorted((k, str(v)) for k, v in actions.items()))
    key = (mask_sig, pmask_np.shape[0])
    if key not in _PROGRAM_CACHE:
        _PROGRAM_CACHE[key] = _build_program(actions, pmask_np.shape[0])
    nc = _PROGRAM_CACHE[key]

    asc = np.asarray(attn_scale, np.float32)
    wqkv_eff = asc[:, None] * np.asarray(wqkv_w, np.float32)
    wq, wk, wv = (wqkv_eff[:, :DIM], wqkv_eff[:, DIM:2 * DIM],
                  wqkv_eff[:, 2 * DIM:])
    out_w_bf = _bf16(out_w)
    l1_bf = _bf16(np.asarray(ffn_scale, np.float32)[:, None]
                  * np.asarray(lin1_w, np.float32))
    l2_bf = _bf16(np.asarray(lin2_w, np.float32)
                  * np.asarray(ln_g, np.float32)[:, None])
    cosT, sinT = _rope_tables()
    eye = np.eye(128, dtype=ml_dtypes.bfloat16)

    x2 = np.ascontiguousarray(x.reshape(R, DIM))
    in_maps = []
    for c in range(NCORES):
        sl = np.concatenate([wq[:, 128 * c:128 * (c + 1)],
                             wk[:, 128 * c:128 * (c + 1)],
                             wv[:, 128 * c:128 * (c + 1)]], axis=1)
        in_maps.append(dict(
            x_own=np.ascontiguousarray(x2[RC * c:RC * (c + 1)]),
            wqkv_sl=_bf16(sl),
            out_w=out_w_bf,
            lin1_w=l1_bf,
            lin2_w=l2_bf,
            cosT=cosT,
            sinT=sinT,
            pmask=pmask_np,
            eye=eye,
        ))

    global _LAST_IN_MAPS
    _LAST_IN_MAPS = in_maps
    res = run_bass_kernel_spmd(nc, in_maps, core_ids=list(range(NCORES)))
    y = np.concatenate([res.results[c]["y_own"] for c in range(NCORES)], axis=0)
    return y.reshape(B, L, DIM).astype(np.float32)


# revision 13
# speedup vs baseline: 1.1134x; 1.1134x over previous
"""Trainium2 Bass kernel for nn_Block_30313879175568 (dense transformer block).

Sharding: head-parallel attention (2 heads/core on 8 cores) + row-parallel
FFN/out-proj (1024 rows/core). Collectives: AllGather of rms-normed
activations (bf16, transposed layout), AllToAll of attention outputs
(heads -> rows). All matmul operands bf16 (fp32 PSUM accumulation); vector
math fp32.

Self-contained: imports only installed packages (concourse et al.) + numpy.
"""

import numpy as np
import ml_dtypes

import concourse.bass as bass  # noqa: F401
import concourse.mybir as mybir
import concourse.tile as tile
from concourse import bacc
from concourse.bass_utils import run_bass_kernel_spmd

BF16 = mybir.dt.bfloat16
F32 = mybir.dt.float32
AF = mybir.ActivationFunctionType
ALU = mybir.AluOpType

B, L, DIM, H, HID = 4, 2048, 1024, 16, 4096
HEAD_DIM = 64
NCORES = 8
R = B * L              # 8192 global rows
RC = R // NCORES       # 1024 rows per core
HPC = H // NCORES      # 2 heads per core
RMS_EPS = 1e-6
LN_EPS = 1e-5
VCOL = 2 * (HEAD_DIM + 1)   # 130: V cols per (batch,ktile) block incl ones
SH = 2 * HEAD_DIM + 2        # 130: a2a shard rows: 2x64 o^T dims + 2 rowsum rows

_PROGRAM_CACHE = {}
_LAST_IN_MAPS = None


# ----------------------------------------------------------------------------
# host-side helpers
# ----------------------------------------------------------------------------

def _bf16(a):
    return np.asarray(a, dtype=np.float32).astype(ml_dtypes.bfloat16)


def _rope_tables():
    half = HEAD_DIM // 2
    inv_freq = 10000.0 ** (-np.arange(0, half, dtype=np.float32) * 2.0 / HEAD_DIM)
    pos = np.arange(L, dtype=np.float32)
    theta = pos[:, None] * inv_freq[None, :]          # [L, 32]
    cos = np.cos(theta).T.astype(np.float32)          # [32, L]
    sin = np.sin(theta).T.astype(np.float32)
    return (np.tile(cos, (4, 1)).copy(), np.tile(sin, (4, 1)).copy())  # [128, L]


def _classify_mask(mask):
    """Split mask^T [k, q] into (16 ktile x 4 qblock) blocks.

    Returns (actions, pmask_np): actions[(kt, qb)] is 'skip' | 'noop' |
    ('mul', idx); pmask_np is [NU, 128, 512] bf16 of exp(mask^T block).
    """
    maskT = np.asarray(mask, dtype=np.float32).T
    actions = {}
    uniq = {}
    tiles = []
    for qb in range(4):
        for kt in range(16):
            blk = maskT[128 * kt:128 * (kt + 1), 512 * qb:512 * (qb + 1)]
            if np.all(blk <= -30.0):
                actions[(kt, qb)] = "skip"
            elif np.all(blk == 0.0):
                actions[(kt, qb)] = "noop"
            else:
                pm = _bf16(np.exp(blk.astype(np.float64)))
                key = pm.tobytes()
                if key not in uniq:
                    uniq[key] = len(tiles)
                    tiles.append(pm)
                actions[(kt, qb)] = ("mul", uniq[key])
    if not tiles:
        tiles = [np.zeros((128, 512), dtype=ml_dtypes.bfloat16)]
    pmask_np = np.stack(tiles, axis=0)
    return actions, pmask_np


def _numpy_fallback(x, mask, attn_scale, wqkv_w, wqkv_b, out_w, out_b,
                    ffn_scale, lin1_w, lin1_b, ln_g, ln_b, lin2_w, lin2_b):
    """Correct (slow) host fallback for configurations the device program
    doesn't support (nonzero biases / fully-masked rows)."""
    from scipy.special import erf

    def rms(t, scale):
        return t / np.sqrt(np.mean(t * t, axis=-1, keepdims=True) + RMS_EPS) * scale

    x = np.asarray(x, np.float64)
    xn = rms(x, attn_scale)
    qkv = xn @ np.asarray(wqkv_w, np.float64) + wqkv_b
    q, k, v = np.split(qkv, 3, axis=-1)
    th = lambda t: t.reshape(B, L, H, HEAD_DIM).transpose(0, 2, 1, 3)
    q, k, v = th(q), th(k), th(v)

    half = HEAD_DIM // 2
    inv_freq = 10000.0 ** (-np.arange(0, half) * 2.0 / HEAD_DIM)
    theta = np.arange(L)[:, None] * inv_freq[None, :]
    cos, sin = np.cos(theta), np.sin(theta)

    def rope(t):
        x1, x2 = t[..., :half], t[..., half:]
        return np.concatenate([x1 * cos - x2 * sin, x1 * sin + x2 * cos], axis=-1)

    q, k = rope(q), rope(k)
    s = np.einsum("bhqd,bhkd->bhqk", q / np.sqrt(HEAD_DIM), k) + np.asarray(mask, np.float64)
    s = s - s.max(axis=-1, keepdims=True)
    p = np.exp(s)
    p /= p.sum(axis=-1, keepdims=True)
    o = np.einsum("bhqk,bhkd->bhqd", p, v)
    o = o.transpose(0, 2, 1, 3).reshape(B, L, DIM)
    h = x + o @ np.asarray(out_w, np.float64) + out_b
    f = rms(h, ffn_scale)
    f = f @ np.asarray(lin1_w, np.float64) + lin1_b
    f = 0.5 * f * (1.0 + erf(f / np.sqrt(2.0)))
    mu = f.mean(axis=-1, keepdims=True)
    var = f.var(axis=-1, keepdims=True)
    f = (f - mu) / np.sqrt(var + LN_EPS) * ln_g + ln_b
    out = h + f @ np.asarray(lin2_w, np.float64) + lin2_b
    return out.astype(np.float32)


# ----------------------------------------------------------------------------
# device program
# ----------------------------------------------------------------------------

def _rms_rstd(nc, stats, t, eps):
    """1/sqrt(mean(t^2, free) + eps) for a [128, D] f32 tile (D mult of 512)."""
    D = t.shape[1]
    g = D // 512
    st = stats.tile([128, g, 6], F32, tag="rmsst", name="rmsst")
    tv = t[:].rearrange("p (g f) -> p g f", g=g)
    for i in range(g):
        nc.vector.bn_stats(st[:, i, :], tv[:, i, :])
    mv = stats.tile([128, 2], F32, tag="rmsmv", name="rmsmv")
    nc.vector.bn_aggr(mv[:], st[:])
    ms = stats.tile([128, 1], F32, tag="rmsms", name="rmsms")
    nc.vector.tensor_scalar(ms[:], mv[:, 0:1], mv[:, 0:1], None, ALU.mult)
    nc.vector.tensor_add(ms[:], ms[:], mv[:, 1:2])
    std = stats.tile([128, 1], F32, tag="rmssd", name="rmssd")
    nc.scalar.activation(std[:], ms[:], AF.Sqrt, bias=eps, scale=1.0)
    rstd = stats.tile([128, 1], F32, tag="rmsrs", name="rmsrs")
    nc.vector.reciprocal(rstd[:], std[:])
    return rstd


def _register_const(nc, value, dtype=F32):
    t = nc.alloc_sbuf_tensor(f"const-{dtype.name}-{value}", [128, 1], dtype)
    nc.gpsimd.memset(t.ap(), value)
    nc.const_aps.aps[(dtype, value)] = t.ap()


def _build_program(actions, n_pmask):
    nc = bacc.Bacc("TRN2", target_bir_lowering=False, debug=False,
                   num_devices=NCORES)
    _register_const(nc, RMS_EPS)
    _register_const(nc, LN_EPS)
    nc.all_engine_barrier()

    x_in = nc.dram_tensor("x_own", [RC, DIM], F32, kind="ExternalInput")
    wqkv_in = nc.dram_tensor("wqkv_sl", [DIM, 3 * 128], BF16, kind="ExternalInput")
    outw_in = nc.dram_tensor("out_w", [DIM, DIM], BF16, kind="ExternalInput")
    l1w_in = nc.dram_tensor("lin1_w", [DIM, HID], BF16, kind="ExternalInput")
    l2w_in = nc.dram_tensor("lin2_w", [HID, DIM], BF16, kind="ExternalInput")
    cos_in = nc.dram_tensor("cosT", [128, L], F32, kind="ExternalInput")
    sin_in = nc.dram_tensor("sinT", [128, L], F32, kind="ExternalInput")
    pm_in = nc.dram_tensor("pmask", [n_pmask, 128, 512], BF16, kind="ExternalInput")
    eye_in = nc.dram_tensor("eye", [128, 128], BF16, kind="ExternalInput")
    y_out = nc.dram_tensor("y_own", [RC, DIM], F32, kind="ExternalOutput")

    with tile.TileContext(nc) as tc:
        _emit(nc, tc, x_in, wqkv_in, outw_in, l1w_in, l2w_in, cos_in, sin_in,
              pm_in, eye_in, y_out, actions, n_pmask)

    nc.compile()
    return nc


def _emit(nc, tc, x_in, wqkv_in, outw_in, l1w_in, l2w_in, cos_in, sin_in,
          pm_in, eye_in, y_out, actions, n_pmask):
    with (
        tc.tile_pool(name="dram", bufs=1, space="DRAM") as dram,
        tc.tile_pool(name="base", bufs=1) as base,
        tc.tile_pool(name="work", bufs=4) as work,
        tc.tile_pool(name="stats", bufs=4) as stats,
    ):
        eye = base.tile([128, 128], BF16)
        nc.sync.dma_start(eye[:], eye_in.ap())

        ag_in0 = dram.tile([DIM, RC // 2], BF16)
        ag_in1 = dram.tile([DIM, RC // 2], BF16)
        ag_out0 = dram.tile([NCORES * DIM, RC // 2], BF16, addr_space="Shared")
        ag_out1 = dram.tile([NCORES * DIM, RC // 2], BF16, addr_space="Shared")
        a2a_in = dram.tile([NCORES * SH, RC], BF16)
        a2a_out = dram.tile([NCORES * SH, RC], BF16)

        with tc.tile_pool(name="xp", bufs=1) as xp:
            x_t = [xp.tile([128, DIM], F32, tag=f"x{rt}", name=f"x{rt}")
                   for rt in range(8)]

            # ---------------- phase 1: load x, rms-norm, transpose -> ag_in
            with (
                tc.tile_pool(name="ps1", bufs=2, space="PSUM") as ps1,
                tc.tile_pool(name="p1", bufs=3) as p1,
            ):
                for rt in range(8):
                    nc.sync.dma_start(x_t[rt][:], x_in.ap()[128 * rt:128 * (rt + 1), :])
                    rstd = _rms_rstd(nc, stats, x_t[rt], RMS_EPS)
                    xn = p1.tile([128, DIM], BF16, tag="xn", name="xn")
                    nc.scalar.activation(xn[:], x_t[rt][:], AF.Copy, bias=0.0,
                                         scale=rstd[:])
                    for fc in range(8):
                        ps = ps1.tile([128, 128], BF16, tag="tr", name="tr")
                        nc.tensor.transpose(ps[:], xn[:, 128 * fc:128 * (fc + 1)], eye[:])
                        ev = p1.tile([128, 128], BF16, tag="ev", name="ev")
                        nc.vector.tensor_copy(ev[:], ps[:])
                        agd = ag_in0 if rt < 4 else ag_in1
                        lrt = rt % 4
                        nc.sync.dma_start(
                            agd[128 * fc:128 * (fc + 1), 128 * lrt:128 * (lrt + 1)],
                            ev[:])
                    if rt == 3:
                        nc.gpsimd.collective_compute(
                            "AllGather", ALU.bypass,
                            replica_groups=[list(range(NCORES))],
                            ins=[ag_in0[:].opt()], outs=[ag_out0[:].opt()])
                    if rt == 7:
                        nc.gpsimd.collective_compute(
                            "AllGather", ALU.bypass,
                            replica_groups=[list(range(NCORES))],
                            ins=[ag_in1[:].opt()], outs=[ag_out1[:].opt()])

            with (
                tc.tile_pool(name="attn", bufs=1) as attn,
                tc.tile_pool(name="cst", bufs=1) as cst,
            ):
                cosT = cst.tile([128, L], F32)
                sinT = cst.tile([128, L], F32)
                nc.sync.dma_start(cosT[:], cos_in.ap())
                nc.sync.dma_start(sinT[:], sin_in.ap())
                pm_t = []
                for i in range(n_pmask):
                    t = cst.tile([128, 512], BF16, tag=f"pm{i}", name=f"pm{i}")
                    nc.sync.dma_start(t[:], pm_in.ap()[i, :, :])
                    pm_t.append(t)
                w_t = []
                for kc in range(8):
                    t = cst.tile([128, 3 * 128], BF16, tag=f"w{kc}", name=f"w{kc}")
                    nc.sync.dma_start(t[:], wqkv_in.ap()[128 * kc:128 * (kc + 1), :])
                    w_t.append(t)

                QT = attn.tile([128, R], BF16)     # [2heads x 64dim, global row]
                KT = attn.tile([128, R], BF16)
                VT = attn.tile([128, R], BF16)
                V_sb = attn.tile([128, 64 * VCOL], BF16)
                nc.vector.memset(V_sb[:], 1.0)

                # ------------ phase 3: QKV + RoPE
                with (
                    tc.tile_pool(name="ps3", bufs=2, space="PSUM") as ps3,
                    tc.tile_pool(name="p3", bufs=3) as p3,
                ):
                    for blk in [0, 2, 4, 6, 8, 10, 12, 14,
                                1, 3, 5, 7, 9, 11, 13, 15]:
                        cb, half = blk // 2, blk % 2
                        ag_src = ag_out0 if half == 0 else ag_out1
                        pos0 = 512 * (blk % 4)
                        g0 = 512 * blk
                        xnt = []
                        for kc in range(8):
                            t = p3.tile([128, 512], BF16, tag="xnt", name="xnt",
                                        bufs=12)
                            nc.sync.dma_start(
                                t[:],
                                ag_src[DIM * cb + 128 * kc:DIM * cb + 128 * (kc + 1),
                                       :])
                            xnt.append(t)
                        psX1 = ps3.tile([128, 512], F32, tag="q", name="psx1")
                        psX2 = ps3.tile([128, 512], F32, tag="k", name="psx2")
                        psV = ps3.tile([128, 512], F32, tag="v", name="psv")
                        for kc in range(8):
                            st, sp = kc == 0, kc == 7
                            nc.tensor.matmul(psX1[:], w_t[kc][:, 0:128], xnt[kc][:],
                                             start=st, stop=sp)
                            nc.tensor.matmul(psX2[:], w_t[kc][:, 128:256], xnt[kc][:],
                                             start=st, stop=sp)
                            nc.tensor.matmul(psV[:], w_t[kc][:, 256:384], xnt[kc][:],
                                             start=st, stop=sp)
                        # full-width rope: psX1 rows = [qx1A qx1B kx1A kx1B],
                        # psX2 rows = [qx2A qx2B kx2A kx2B]
                        cs = cosT[:, pos0:pos0 + 512]
                        sn = sinT[:, pos0:pos0 + 512]
                        t1 = work.tile([128, 512], F32, tag="r1", name="r1")
                        t2 = work.tile([128, 512], F32, tag="r2", name="r2")
                        o1 = p3.tile([128, 512], BF16, tag="o1", name="o1")
                        o2 = p3.tile([128, 512], BF16, tag="o2", name="o2")
                        nc.vector.tensor_mul(t1[:], psX1[:], cs)
                        nc.vector.tensor_mul(t2[:], psX2[:], sn)
                        nc.vector.tensor_sub(o1[:], t1[:], t2[:])
                        nc.vector.tensor_mul(t1[:], psX1[:], sn)
                        nc.vector.tensor_mul(t2[:], psX2[:], cs)
                        nc.vector.tensor_add(o2[:], t1[:], t2[:])
                        # reassemble into QT/KT ([headA d0-63, headB d0-63])
                        for src_t, d0, dst, dr in (
                            (o1, 0, QT, 0), (o2, 0, QT, 32),
                            (o1, 32, QT, 64), (o2, 32, QT, 96),
                            (o1, 64, KT, 0), (o2, 64, KT, 32),
                            (o1, 96, KT, 64), (o2, 96, KT, 96),
                        ):
                            nc.sync.dma_start(dst[dr:dr + 32, g0:g0 + 512],
                                              src_t[d0:d0 + 32, :])
                        nc.vector.tensor_copy(VT[:, g0:g0 + 512], psV[:])

                    # ------------ phase 4: V^T -> row-major V blocks (+ones)
                    for bt in range(64):
                        pst = ps3.tile([128, 128], BF16, tag="tr", name="tr")
                        nc.tensor.transpose(pst[:], VT[:, 128 * bt:128 * (bt + 1)],
                                            eye[:])
                        nc.vector.tensor_copy(V_sb[:, VCOL * bt:VCOL * bt + 64],
                                              pst[:, 0:64])
                        nc.vector.tensor_copy(V_sb[:, VCOL * bt + 65:VCOL * bt + 129],
                                              pst[:, 64:128])

                # ------------ phase 5: attention per (batch, head, qblock)
                with (
                    tc.tile_pool(name="ps5s", bufs=4, space="PSUM") as ps5s,
                    tc.tile_pool(name="ps5o", bufs=3, space="PSUM") as ps5o,
                    tc.tile_pool(name="p5", bufs=3) as p5,
                ):
                    for b in range(B):
                        for h in range(HPC):
                            hr = 64 * h
                            for qb in range(4):
                                q0 = 2048 * b + 512 * qb
                                act = [(kt, actions[(kt, qb)]) for kt in range(16)
                                       if actions[(kt, qb)] != "skip"]
                                psO = ps5o.tile([65, 512], F32, tag="o", name="pso")
                                pts = []
                                for kt, a in act:
                                    k0 = 2048 * b + 128 * kt
                                    psS = ps5s.tile([128, 512], F32, tag="s",
                                                    name="pss")
                                    nc.tensor.matmul(
                                        psS[:], KT[hr:hr + 64, k0:k0 + 128],
                                        QT[hr:hr + 64, q0:q0 + 512],
                                        start=True, stop=True)
                                    pt = p5.tile([128, 512], BF16, tag="pt",
                                                 name="pt", bufs=18)
                                    nc.scalar.activation(pt[:], psS[:], AF.Exp,
                                                         bias=0.0, scale=0.125)
                                    if a != "noop":
                                        nc.vector.tensor_mul(pt[:], pt[:],
                                                             pm_t[a[1]][:])
                                    pts.append((kt, pt))
                                for i, (kt, pt) in enumerate(pts):
                                    bt = 16 * b + kt
                                    nc.tensor.matmul(
                                        psO[:],
                                        V_sb[:, VCOL * bt + 65 * h:
                                             VCOL * bt + 65 * h + 65],
                                        pt[:],
                                        start=(i == 0), stop=(i == len(pts) - 1))
                                ot65 = p5.tile([65, 512], BF16, tag="ot",
                                               name="ot65")
                                nc.vector.tensor_copy(ot65[:], psO[:, :])
                                j = (2048 * b + 512 * qb) // RC
                                loc0 = 2048 * b + 512 * qb - RC * j
                                nc.sync.dma_start(
                                    a2a_in[SH * j + 65 * h:SH * j + 65 * h + 65,
                                           loc0:loc0 + 512],
                                    ot65[:])

            # ---------------- phase 6: AllToAll o^T heads -> rows
            nc.gpsimd.collective_compute(
                "AllToAll", ALU.bypass,
                replica_groups=[list(range(NCORES))],
                ins=[a2a_in[:].opt()], outs=[a2a_out[:].opt()])

            # ---------------- phase 7: out-proj + residual -> h
            with tc.tile_pool(name="hp", bufs=1) as hp:
                h_t = [hp.tile([128, DIM], F32, tag=f"h{rt}", name=f"h{rt}")
                       for rt in range(8)]
                with (
                    tc.tile_pool(name="ps7", bufs=1, space="PSUM") as ps7,
                    tc.tile_pool(name="p7", bufs=1) as p7,
                    tc.tile_pool(name="p7w", bufs=6) as p7w,
                ):
                    oT = []
                    for kc in range(8):
                        t = p7.tile([128, RC], BF16, tag=f"ot{kc}", name=f"oT{kc}")
                        nc.sync.dma_start(t[0:64, :],
                                          a2a_out[SH * kc:SH * kc + 64, :])
                        nc.sync.dma_start(t[64:128, :],
                                          a2a_out[SH * kc + 65:SH * kc + 129, :])
                        oT.append(t)
                    rs_sb = p7.tile([16, RC], BF16, tag="rs", name="rs_sb")
                    for kc in range(8):
                        for hh in range(2):
                            nc.sync.dma_start(
                                rs_sb[2 * kc + hh:2 * kc + hh + 1, :],
                                a2a_out[SH * kc + 65 * hh + 64:
                                        SH * kc + 65 * hh + 65, :])
                    rs_rec = p7.tile([16, RC], F32, tag="rsr", name="rs_rec")
                    nc.vector.reciprocal(rs_rec[:], rs_sb[:])
                    rs_rb = p7.tile([16, RC], BF16, tag="rsb", name="rs_rb")
                    nc.vector.tensor_copy(rs_rb[:], rs_rec[:])
                    for kc in range(8):
                        bc = p7w.tile([128, RC], BF16, tag="bc", name="bc",
                                      bufs=3)
                        for hh in range(2):
                            r1 = p7w.tile([1, RC], BF16, tag="r1b", name="r1b",
                                          bufs=3)
                            nc.sync.dma_start(
                                r1[:], rs_rb[2 * kc + hh:2 * kc + hh + 1, :])
                            if hh == 0:
                                nc.gpsimd.partition_broadcast(bc[0:64, :], r1[:])
                            else:
                                tb = p7w.tile([64, RC], BF16, tag="tb",
                                              name="tb", bufs=3)
                                nc.gpsimd.partition_broadcast(tb[:], r1[:])
                                nc.sync.dma_start(bc[64:128, :], tb[:])
                        nc.vector.tensor_mul(oT[kc][:], oT[kc][:], bc[:])
                    for nb in range(2):
                        pss = [ps7.tile([128, 512], F32, tag=f"mm{rt % 4}",
                                        name="psmm", bufs=2) for rt in range(8)]
                        for kc in range(8):
                            w = p7w.tile([128, 512], BF16, tag="ow", name="ow")
                            nc.sync.dma_start(
                                w[:], outw_in.ap()[128 * kc:128 * (kc + 1),
                                                   512 * nb:512 * (nb + 1)])
                            for rt in range(8):
                                nc.tensor.matmul(pss[rt][:],
                                                 oT[kc][:, 128 * rt:128 * (rt + 1)],
                                                 w[:], start=(kc == 0),
                                                 stop=(kc == 7))
                        for rt in range(8):
                            nc.vector.tensor_add(
                                h_t[rt][:, 512 * nb:512 * (nb + 1)], pss[rt][:],
                                x_t[rt][:, 512 * nb:512 * (nb + 1)])

                # ------------ phase 8: FFN (row-local), two halves of 512 rows
                with (
                    tc.tile_pool(name="ps8", bufs=1, space="PSUM") as ps8,
                    tc.tile_pool(name="ps8t", bufs=2, space="PSUM") as ps8t,
                    tc.tile_pool(name="p8", bufs=1) as p8,
                    tc.tile_pool(name="p8w", bufs=6) as p8w,
                    tc.tile_pool(name="p8s", bufs=3) as p8s,
                ):
                    for half in range(2):
                        # rms-norm h -> fn (bf16) -> transpose -> fnT
                        fnT = [p8.tile([128, 512], BF16, tag=f"fnT{fc}",
                                       name=f"fnT{fc}") for fc in range(8)]
                        for rt2 in range(4):
                            rt = 4 * half + rt2
                            rstd = _rms_rstd(nc, stats, h_t[rt], RMS_EPS)
                            fn = p8s.tile([128, DIM], BF16, tag="fn", name="fn")
                            nc.scalar.activation(fn[:], h_t[rt][:], AF.Copy,
                                                 bias=0.0, scale=rstd[:])
                            for fc in range(8):
                                ps = ps8t.tile([128, 128], BF16, tag="tr", name="tr")
                                nc.tensor.transpose(
                                    ps[:], fn[:, 128 * fc:128 * (fc + 1)], eye[:])
                                nc.vector.tensor_copy(
                                    fnT[fc][:, 128 * rt2:128 * (rt2 + 1)], ps[:])
                        # lin1 + GELU -> g [4 x 8 tiles of [128,512] bf16]
                        g_t = [[p8.tile([128, 512], BF16, tag=f"g{rt2}_{hb}",
                                        name=f"g{rt2}_{hb}")
                                for hb in range(8)] for rt2 in range(4)]
                        for hb in range(8):
                            pss = [ps8.tile([128, 512], F32, tag=f"mm{rt2}",
                                            name="psmm", bufs=1)
                                   for rt2 in range(4)]
                            for fc in range(8):
                                w = p8w.tile([128, 512], BF16, tag="l1w", name="l1w")
                                nc.sync.dma_start(
                                    w[:], l1w_in.ap()[128 * fc:128 * (fc + 1),
                                                      512 * hb:512 * (hb + 1)])
                                for rt2 in range(4):
                                    nc.tensor.matmul(
                                        pss[rt2][:],
                                        fnT[fc][:, 128 * rt2:128 * (rt2 + 1)],
                                        w[:], start=(fc == 0), stop=(fc == 7))
                            for rt2 in range(4):
                                nc.scalar.activation(g_t[rt2][hb][:], pss[rt2][:],
                                                     AF.Gelu)
                        # LayerNorm stats over hid (4096) per row
                        ab = []
                        for rt2 in range(4):
                            st = stats.tile([128, 8, 6], F32, tag="lnst",
                                            name="lnst")
                            for hb in range(8):
                                nc.vector.bn_stats(st[:, hb, :], g_t[rt2][hb][:])
                            mv = stats.tile([128, 2], F32, tag="lnmv", name="lnmv")
                            nc.vector.bn_aggr(mv[:], st[:])
                            std = stats.tile([128, 1], F32, tag="lnsd", name="lnsd")
                            nc.scalar.activation(std[:], mv[:, 1:2], AF.Sqrt,
                                                 bias=LN_EPS, scale=1.0)
                            rstd = stats.tile([128, 1], F32, tag="lnrs",
                                              name="lnrs")
                            nc.vector.reciprocal(rstd[:], std[:])
                            nmr = stats.tile([128, 1], F32, tag="lnnm", name="lnnm")
                            nc.vector.tensor_scalar(nmr[:], rstd[:], mv[:, 0:1],
                                                    -1.0, ALU.mult, ALU.mult)
                            ab.append((rstd, nmr))
                        # normalize + transpose -> gnT [32 tiles of [128,512]]
                        gnT = [p8.tile([128, 512], BF16, tag=f"gnT{hc}",
                                       name=f"gnT{hc}") for hc in range(32)]
                        for rt2 in range(4):
                            rstd, nmr = ab[rt2]
                            for hb in range(8):
                                gn = p8s.tile([128, 512], BF16, tag="gn", name="gn")
                                nc.vector.tensor_scalar(gn[:], g_t[rt2][hb][:],
                                                        rstd[:], nmr[:],
                                                        ALU.mult, ALU.add)
                                for j in range(4):
                                    ps = ps8t.tile([128, 128], BF16, tag="tr",
                                                   name="tr")
                                    nc.tensor.transpose(
                                        ps[:], gn[:, 128 * j:128 * (j + 1)], eye[:])
                                    nc.vector.tensor_copy(
                                        gnT[4 * hb + j][:, 128 * rt2:128 * (rt2 + 1)],
                                        ps[:])
                        # lin2 + residual -> y
                        for nb in range(2):
                            pss = [ps8.tile([128, 512], F32, tag=f"mm{rt2}",
                                            name="psmm", bufs=1)
                                   for rt2 in range(4)]
                            for hc in range(32):
                                w = p8w.tile([128, 512], BF16, tag="l2w", name="l2w")
                                nc.sync.dma_start(
                                    w[:], l2w_in.ap()[128 * hc:128 * (hc + 1),
                                                      512 * nb:512 * (nb + 1)])
                                for rt2 in range(4):
                                    nc.tensor.matmul(
                                        pss[rt2][:],
                                        gnT[hc][:, 128 * rt2:128 * (rt2 + 1)],
                                        w[:], start=(hc == 0), stop=(hc == 31))
                            for rt2 in range(4):
                                rt = 4 * half + rt2
                                yt = p8s.tile([128, 512], F32, tag="yt", name="yt")
                                nc.vector.tensor_add(
                                    yt[:], pss[rt2][:],
                                    h_t[rt][:, 512 * nb:512 * (nb + 1)])
                                nc.sync.dma_start(
                                    y_out.ap()[128 * rt:128 * (rt + 1),
                                               512 * nb:512 * (nb + 1)],
                                    yt[:])


# ----------------------------------------------------------------------------
# entry point
# ----------------------------------------------------------------------------

def kernel(x, mask, attn_scale, wqkv_w, wqkv_b, out_w, out_b,
           ffn_scale, lin1_w, lin1_b, ln_g, ln_b, lin2_w, lin2_b):
    x = np.asarray(x, np.float32)
    mask = np.asarray(mask, np.float32)

    lin2_b_eff = (np.asarray(lin2_b, np.float32)
                  + np.asarray(ln_b, np.float32) @ np.asarray(lin2_w, np.float32))
    if np.any(wqkv_b) or np.any(out_b) or np.any(lin1_b) or np.any(lin2_b_eff):
        return _numpy_fallback(x, mask, attn_scale, wqkv_w, wqkv_b, out_w, out_b,
                               ffn_scale, lin1_w, lin1_b, ln_g, ln_b, lin2_w,
                               lin2_b)

    actions, pmask_np = _classify_mask(mask)
    for qb in range(4):
        if all(actions[(kt, qb)] == "skip" for kt in range(16)):
            return _numpy_fallback(x, mask, attn_scale, wqkv_w, wqkv_b, out_w,
                                   out_b, ffn_scale, lin1_w, lin1_b, ln_g, ln_b,
                                   lin2_w, lin2_b)

    mask_sig = tuple(sorted((k, str(v)) for k, v in actions.items()))
    key = (mask_sig, pmask_np.shape[0])
    if key not in _PROGRAM_CACHE:
        _PROGRAM_CACHE[key] = _build_program(actions, pmask_np.shape[0])
    nc = _PROGRAM_CACHE[key]

    asc = np.asarray(attn_scale, np.float32)
    wqkv_eff = asc[:, None] * np.asarray(wqkv_w, np.float32)
    wq, wk, wv = (wqkv_eff[:, :DIM], wqkv_eff[:, DIM:2 * DIM],
                  wqkv_eff[:, 2 * DIM:])
    out_w_bf = _bf16(out_w)
    l1_bf = _bf16(np.asarray(ffn_scale, np.float32)[:, None]
                  * np.asarray(lin1_w, np.float32))
    l2_bf = _bf16(np.asarray(lin2_w, np.float32)
                  * np.asarray(ln_g, np.float32)[:, None])
    cosT, sinT = _rope_tables()
    eye = np.eye(128, dtype=ml_dtypes.bfloat16)

    x2 = np.ascontiguousarray(x.reshape(R, DIM))
    in_maps = []
    for c in range(NCORES):
        hA, hB = 2 * c, 2 * c + 1
        qA, qB = wq[:, 64 * hA:64 * hA + 64], wq[:, 64 * hB:64 * hB + 64]
        kA, kB = wk[:, 64 * hA:64 * hA + 64], wk[:, 64 * hB:64 * hB + 64]
        # interleaved for full-width rope: [qx1A qx1B kx1A kx1B | x2... | vA vB]
        sl = np.concatenate([qA[:, :32], qB[:, :32], kA[:, :32], kB[:, :32],
                             qA[:, 32:], qB[:, 32:], kA[:, 32:], kB[:, 32:],
                             wv[:, 128 * c:128 * (c + 1)]], axis=1)
        in_maps.append(dict(
            x_own=np.ascontiguousarray(x2[RC * c:RC * (c + 1)]),
            wqkv_sl=_bf16(sl),
            out_w=out_w_bf,
            lin1_w=l1_bf,
            lin2_w=l2_bf,
            cosT=cosT,
            sinT=sinT,
            pmask=pmask_np,
            eye=eye,
        ))

    global _LAST_IN_MAPS
    _LAST_IN_MAPS = in_maps
    res = run_bass_kernel_spmd(nc, in_maps, core_ids=list(range(NCORES)))
    y = np.concatenate([res.results[c]["y_own"] for c in range(NCORES)], axis=0)
    return y.reshape(B, L, DIM).astype(np.float32)


# revision 14
# speedup vs baseline: 1.1390x; 1.0230x over previous
"""Trainium2 Bass kernel for nn_Block_30313879175568 (dense transformer block).

Sharding: head-parallel attention (2 heads/core on 8 cores) + row-parallel
FFN/out-proj (1024 rows/core). Collectives: AllGather of rms-normed
activations (bf16, transposed layout), AllToAll of attention outputs
(heads -> rows). All matmul operands bf16 (fp32 PSUM accumulation); vector
math fp32.

Self-contained: imports only installed packages (concourse et al.) + numpy.
"""

import numpy as np
import ml_dtypes

import concourse.bass as bass  # noqa: F401
import concourse.mybir as mybir
import concourse.tile as tile
from concourse import bacc
from concourse.bass_utils import run_bass_kernel_spmd

BF16 = mybir.dt.bfloat16
F32 = mybir.dt.float32
AF = mybir.ActivationFunctionType
ALU = mybir.AluOpType

B, L, DIM, H, HID = 4, 2048, 1024, 16, 4096
HEAD_DIM = 64
NCORES = 8
R = B * L              # 8192 global rows
RC = R // NCORES       # 1024 rows per core
HPC = H // NCORES      # 2 heads per core
RMS_EPS = 1e-6
LN_EPS = 1e-5
VCOL = 2 * (HEAD_DIM + 1)   # 130: V cols per (batch,ktile) block incl ones
SH = 2 * HEAD_DIM + 2        # 130: a2a shard rows: 2x64 o^T dims + 2 rowsum rows

_PROGRAM_CACHE = {}
_LAST_IN_MAPS = None


# ----------------------------------------------------------------------------
# host-side helpers
# ----------------------------------------------------------------------------

def _bf16(a):
    return np.asarray(a, dtype=np.float32).astype(ml_dtypes.bfloat16)


def _rope_tables():
    half = HEAD_DIM // 2
    inv_freq = 10000.0 ** (-np.arange(0, half, dtype=np.float32) * 2.0 / HEAD_DIM)
    pos = np.arange(L, dtype=np.float32)
    theta = pos[:, None] * inv_freq[None, :]          # [L, 32]
    cos = np.cos(theta).T.astype(np.float32)          # [32, L]
    sin = np.sin(theta).T.astype(np.float32)
    return (np.tile(cos, (4, 1)).copy(), np.tile(sin, (4, 1)).copy())  # [128, L]


def _classify_mask(mask):
    """Split mask^T [k, q] into (16 ktile x 4 qblock) blocks.

    Returns (actions, pmask_np): actions[(kt, qb)] is 'skip' | 'noop' |
    ('mul', idx); pmask_np is [NU, 128, 512] bf16 of exp(mask^T block).
    """
    maskT = np.asarray(mask, dtype=np.float32).T
    actions = {}
    uniq = {}
    tiles = []
    for qb in range(4):
        for kt in range(16):
            blk = maskT[128 * kt:128 * (kt + 1), 512 * qb:512 * (qb + 1)]
            if np.all(blk <= -30.0):
                actions[(kt, qb)] = "skip"
            elif np.all(blk == 0.0):
                actions[(kt, qb)] = "noop"
            else:
                pm = _bf16(np.exp(blk.astype(np.float64)))
                key = pm.tobytes()
                if key not in uniq:
                    uniq[key] = len(tiles)
                    tiles.append(pm)
                actions[(kt, qb)] = ("mul", uniq[key])
    if not tiles:
        tiles = [np.zeros((128, 512), dtype=ml_dtypes.bfloat16)]
    pmask_np = np.stack(tiles, axis=0)
    return actions, pmask_np


def _numpy_fallback(x, mask, attn_scale, wqkv_w, wqkv_b, out_w, out_b,
                    ffn_scale, lin1_w, lin1_b, ln_g, ln_b, lin2_w, lin2_b):
    """Correct (slow) host fallback for configurations the device program
    doesn't support (nonzero biases / fully-masked rows)."""
    from scipy.special import erf

    def rms(t, scale):
        return t / np.sqrt(np.mean(t * t, axis=-1, keepdims=True) + RMS_EPS) * scale

    x = np.asarray(x, np.float64)
    xn = rms(x, attn_scale)
    qkv = xn @ np.asarray(wqkv_w, np.float64) + wqkv_b
    q, k, v = np.split(qkv, 3, axis=-1)
    th = lambda t: t.reshape(B, L, H, HEAD_DIM).transpose(0, 2, 1, 3)
    q, k, v = th(q), th(k), th(v)

    half = HEAD_DIM // 2
    inv_freq = 10000.0 ** (-np.arange(0, half) * 2.0 / HEAD_DIM)
    theta = np.arange(L)[:, None] * inv_freq[None, :]
    cos, sin = np.cos(theta), np.sin(theta)

    def rope(t):
        x1, x2 = t[..., :half], t[..., half:]
        return np.concatenate([x1 * cos - x2 * sin, x1 * sin + x2 * cos], axis=-1)

    q, k = rope(q), rope(k)
    s = np.einsum("bhqd,bhkd->bhqk", q / np.sqrt(HEAD_DIM), k) + np.asarray(mask, np.float64)
    s = s - s.max(axis=-1, keepdims=True)
    p = np.exp(s)
    p /= p.sum(axis=-1, keepdims=True)
    o = np.einsum("bhqk,bhkd->bhqd", p, v)
    o = o.transpose(0, 2, 1, 3).reshape(B, L, DIM)
    h = x + o @ np.asarray(out_w, np.float64) + out_b
    f = rms(h, ffn_scale)
    f = f @ np.asarray(lin1_w, np.float64) + lin1_b
    f = 0.5 * f * (1.0 + erf(f / np.sqrt(2.0)))
    mu = f.mean(axis=-1, keepdims=True)
    var = f.var(axis=-1, keepdims=True)
    f = (f - mu) / np.sqrt(var + LN_EPS) * ln_g + ln_b
    out = h + f @ np.asarray(lin2_w, np.float64) + lin2_b
    return out.astype(np.float32)


# ----------------------------------------------------------------------------
# device program
# ----------------------------------------------------------------------------

def _rms_rstd(nc, scratch, stats, t, eps):
    """1/sqrt(mean(t^2, free) + eps) for a [128, D] f32 tile, via ACT."""
    D = t.shape[1]
    sq = scratch.tile([128, D], BF16, tag="sq", name="sq")
    ssq = stats.tile([128, 1], F32, tag="ssq", name="ssq")
    nc.scalar.activation(sq[:], t[:], AF.Square, accum_out=ssq[:])
    std = stats.tile([128, 1], F32, tag="rmssd", name="rmssd")
    nc.scalar.activation(std[:], ssq[:], AF.Sqrt, bias=eps, scale=1.0 / D)
    rstd = stats.tile([128, 1], F32, tag="rmsrs", name="rmsrs")
    nc.vector.reciprocal(rstd[:], std[:])
    return rstd


def _register_const(nc, value, dtype=F32):
    t = nc.alloc_sbuf_tensor(f"const-{dtype.name}-{value}", [128, 1], dtype)
    nc.gpsimd.memset(t.ap(), value)
    nc.const_aps.aps[(dtype, value)] = t.ap()


def _build_program(actions, n_pmask):
    nc = bacc.Bacc("TRN2", target_bir_lowering=False, debug=False,
                   num_devices=NCORES)
    _register_const(nc, RMS_EPS)
    _register_const(nc, LN_EPS)
    nc.all_engine_barrier()

    x_in = nc.dram_tensor("x_own", [RC, DIM], F32, kind="ExternalInput")
    wqkv_in = nc.dram_tensor("wqkv_sl", [DIM, 3 * 128], BF16, kind="ExternalInput")
    outw_in = nc.dram_tensor("out_w", [DIM, DIM], BF16, kind="ExternalInput")
    l1w_in = nc.dram_tensor("lin1_w", [DIM, HID], BF16, kind="ExternalInput")
    l2w_in = nc.dram_tensor("lin2_w", [HID, DIM], BF16, kind="ExternalInput")
    cos_in = nc.dram_tensor("cosT", [128, L], F32, kind="ExternalInput")
    sin_in = nc.dram_tensor("sinT", [128, L], F32, kind="ExternalInput")
    pm_in = nc.dram_tensor("pmask", [n_pmask, 128, 512], BF16, kind="ExternalInput")
    eye_in = nc.dram_tensor("eye", [128, 128], BF16, kind="ExternalInput")
    y_out = nc.dram_tensor("y_own", [RC, DIM], F32, kind="ExternalOutput")

    with tile.TileContext(nc) as tc:
        _emit(nc, tc, x_in, wqkv_in, outw_in, l1w_in, l2w_in, cos_in, sin_in,
              pm_in, eye_in, y_out, actions, n_pmask)

    nc.compile()
    return nc


def _emit(nc, tc, x_in, wqkv_in, outw_in, l1w_in, l2w_in, cos_in, sin_in,
          pm_in, eye_in, y_out, actions, n_pmask):
    with (
        tc.tile_pool(name="dram", bufs=1, space="DRAM") as dram,
        tc.tile_pool(name="base", bufs=1) as base,
        tc.tile_pool(name="work", bufs=4) as work,
        tc.tile_pool(name="stats", bufs=4) as stats,
    ):
        eye = base.tile([128, 128], BF16)
        nc.sync.dma_start(eye[:], eye_in.ap())

        ag_in0 = dram.tile([DIM, RC // 2], BF16)
        ag_in1 = dram.tile([DIM, RC // 2], BF16)
        ag_out0 = dram.tile([NCORES * DIM, RC // 2], BF16, addr_space="Shared")
        ag_out1 = dram.tile([NCORES * DIM, RC // 2], BF16, addr_space="Shared")
        a2a_inA = dram.tile([NCORES * 65, RC], BF16)
        a2a_outA = dram.tile([NCORES * 65, RC], BF16)
        a2a_inB = dram.tile([NCORES * 65, RC], BF16)
        a2a_outB = dram.tile([NCORES * 65, RC], BF16)

        with tc.tile_pool(name="xp", bufs=1) as xp:
            x_t = [xp.tile([128, DIM], F32, tag=f"x{rt}", name=f"x{rt}")
                   for rt in range(8)]

            # ---------------- phase 1: load x, rms-norm, transpose -> ag_in
            with (
                tc.tile_pool(name="ps1", bufs=2, space="PSUM") as ps1,
                tc.tile_pool(name="p1", bufs=3) as p1,
            ):
                for rt in range(8):
                    nc.sync.dma_start(x_t[rt][:], x_in.ap()[128 * rt:128 * (rt + 1), :])
                    rstd = _rms_rstd(nc, p1, stats, x_t[rt], RMS_EPS)
                    xn = p1.tile([128, DIM], BF16, tag="xn", name="xn")
                    nc.scalar.activation(xn[:], x_t[rt][:], AF.Copy, bias=0.0,
                                         scale=rstd[:])
                    for fc in range(8):
                        ps = ps1.tile([128, 128], BF16, tag="tr", name="tr")
                        nc.tensor.transpose(ps[:], xn[:, 128 * fc:128 * (fc + 1)], eye[:])
                        ev = p1.tile([128, 128], BF16, tag="ev", name="ev")
                        nc.vector.tensor_copy(ev[:], ps[:])
                        agd = ag_in0 if rt < 4 else ag_in1
                        lrt = rt % 4
                        nc.sync.dma_start(
                            agd[128 * fc:128 * (fc + 1), 128 * lrt:128 * (lrt + 1)],
                            ev[:])
                    if rt == 3:
                        nc.gpsimd.collective_compute(
                            "AllGather", ALU.bypass,
                            replica_groups=[list(range(NCORES))],
                            ins=[ag_in0[:].opt()], outs=[ag_out0[:].opt()])
                    if rt == 7:
                        nc.gpsimd.collective_compute(
                            "AllGather", ALU.bypass,
                            replica_groups=[list(range(NCORES))],
                            ins=[ag_in1[:].opt()], outs=[ag_out1[:].opt()])

            with (
                tc.tile_pool(name="attn", bufs=1) as attn,
                tc.tile_pool(name="cst", bufs=1) as cst,
            ):
                cosT = cst.tile([128, L], F32)
                sinT = cst.tile([128, L], F32)
                nc.sync.dma_start(cosT[:], cos_in.ap())
                nc.sync.dma_start(sinT[:], sin_in.ap())
                pm_t = []
                for i in range(n_pmask):
                    t = cst.tile([128, 512], BF16, tag=f"pm{i}", name=f"pm{i}")
                    nc.sync.dma_start(t[:], pm_in.ap()[i, :, :])
                    pm_t.append(t)
                w_t = []
                for kc in range(8):
                    t = cst.tile([128, 3 * 128], BF16, tag=f"w{kc}", name=f"w{kc}")
                    nc.sync.dma_start(t[:], wqkv_in.ap()[128 * kc:128 * (kc + 1), :])
                    w_t.append(t)

                QT = attn.tile([128, R], BF16)     # [2heads x 64dim, global row]
                KT = attn.tile([128, R], BF16)
                VT = attn.tile([128, R], BF16)
                V_sb = attn.tile([128, 64 * VCOL], BF16)
                nc.vector.memset(V_sb[:], 1.0)

                # ------------ phase 3: QKV + RoPE
                with (
                    tc.tile_pool(name="ps3", bufs=2, space="PSUM") as ps3,
                    tc.tile_pool(name="p3", bufs=3) as p3,
                ):
                    for blk in [0, 2, 4, 6, 8, 10, 12, 14,
                                1, 3, 5, 7, 9, 11, 13, 15]:
                        cb, half = blk // 2, blk % 2
                        ag_src = ag_out0 if half == 0 else ag_out1
                        pos0 = 512 * (blk % 4)
                        g0 = 512 * blk
                        xnt = []
                        for kc in range(8):
                            t = p3.tile([128, 512], BF16, tag="xnt", name="xnt",
                                        bufs=12)
                            nc.sync.dma_start(
                                t[:],
                                ag_src[DIM * cb + 128 * kc:DIM * cb + 128 * (kc + 1),
                                       :])
                            xnt.append(t)
                        psX1 = ps3.tile([128, 512], F32, tag="q", name="psx1")
                        psX2 = ps3.tile([128, 512], F32, tag="k", name="psx2")
                        psV = ps3.tile([128, 512], F32, tag="v", name="psv")
                        for kc in range(8):
                            st, sp = kc == 0, kc == 7
                            nc.tensor.matmul(psX1[:], w_t[kc][:, 0:128], xnt[kc][:],
                                             start=st, stop=sp)
                            nc.tensor.matmul(psX2[:], w_t[kc][:, 128:256], xnt[kc][:],
                                             start=st, stop=sp)
                            nc.tensor.matmul(psV[:], w_t[kc][:, 256:384], xnt[kc][:],
                                             start=st, stop=sp)
                        # full-width rope: psX1 rows = [qx1A qx1B kx1A kx1B],
                        # psX2 rows = [qx2A qx2B kx2A kx2B]
                        cs = cosT[:, pos0:pos0 + 512]
                        sn = sinT[:, pos0:pos0 + 512]
                        t1 = work.tile([128, 512], F32, tag="r1", name="r1")
                        t2 = work.tile([128, 512], F32, tag="r2", name="r2")
                        o1 = p3.tile([128, 512], BF16, tag="o1", name="o1")
                        o2 = p3.tile([128, 512], BF16, tag="o2", name="o2")
                        nc.vector.tensor_mul(t1[:], psX1[:], cs)
                        nc.vector.tensor_mul(t2[:], psX2[:], sn)
                        nc.vector.tensor_sub(o1[:], t1[:], t2[:])
                        nc.vector.tensor_mul(t1[:], psX1[:], sn)
                        nc.vector.tensor_mul(t2[:], psX2[:], cs)
                        nc.vector.tensor_add(o2[:], t1[:], t2[:])
                        # reassemble into QT/KT ([headA d0-63, headB d0-63])
                        for src_t, d0, dst, dr in (
                            (o1, 0, QT, 0), (o2, 0, QT, 32),
                            (o1, 32, QT, 64), (o2, 32, QT, 96),
                            (o1, 64, KT, 0), (o2, 64, KT, 32),
                            (o1, 96, KT, 64), (o2, 96, KT, 96),
                        ):
                            nc.sync.dma_start(dst[dr:dr + 32, g0:g0 + 512],
                                              src_t[d0:d0 + 32, :])
                        nc.vector.tensor_copy(VT[:, g0:g0 + 512], psV[:])

                    # ------------ phase 4: V^T -> row-major V blocks (+ones)
                    for bt in range(64):
                        pst = ps3.tile([128, 128], BF16, tag="tr", name="tr")
                        nc.tensor.transpose(pst[:], VT[:, 128 * bt:128 * (bt + 1)],
                                            eye[:])
                        nc.vector.tensor_copy(V_sb[:, VCOL * bt:VCOL * bt + 64],
                                              pst[:, 0:64])
                        nc.vector.tensor_copy(V_sb[:, VCOL * bt + 65:VCOL * bt + 129],
                                              pst[:, 64:128])

                # ------------ phase 5: attention per (batch, head, qblock)
                with (
                    tc.tile_pool(name="ps5s", bufs=4, space="PSUM") as ps5s,
                    tc.tile_pool(name="ps5o", bufs=3, space="PSUM") as ps5o,
                    tc.tile_pool(name="p5", bufs=3) as p5,
                ):
                    LOOK = 3
                    for h in range(HPC):
                        hr = 64 * h
                        a2a_dst = a2a_inA if h == 0 else a2a_inB
                        for b in range(B):
                            for qb in range(4):
                                q0 = 2048 * b + 512 * qb
                                act = [(kt, actions[(kt, qb)]) for kt in range(16)
                                       if actions[(kt, qb)] != "skip"]
                                n = len(act)
                                psO = ps5o.tile([65, 512], F32, tag="o", name="pso")
                                pts = []
                                for i in range(n + LOOK):
                                    if i < n:
                                        kt, a = act[i]
                                        k0 = 2048 * b + 128 * kt
                                        psS = ps5s.tile([128, 512], F32, tag="s",
                                                        name="pss")
                                        nc.tensor.matmul(
                                            psS[:], KT[hr:hr + 64, k0:k0 + 128],
                                            QT[hr:hr + 64, q0:q0 + 512],
                                            start=True, stop=True)
                                        pt = p5.tile([128, 512], BF16, tag="pt",
                                                     name="pt", bufs=8)
                                        nc.scalar.activation(pt[:], psS[:], AF.Exp,
                                                             bias=0.0, scale=0.125)
                                        if a != "noop":
                                            nc.vector.tensor_mul(pt[:], pt[:],
                                                                 pm_t[a[1]][:])
                                        pts.append((kt, pt))
                                    j = i - LOOK
                                    if 0 <= j < n:
                                        kt, pt = pts[j]
                                        bt = 16 * b + kt
                                        nc.tensor.matmul(
                                            psO[:],
                                            V_sb[:, VCOL * bt + 65 * h:
                                                 VCOL * bt + 65 * h + 65],
                                            pt[:],
                                            start=(j == 0), stop=(j == n - 1))
                                ot65 = p5.tile([65, 512], BF16, tag="ot",
                                               name="ot65")
                                nc.vector.tensor_copy(ot65[:], psO[:, :])
                                j2 = (2048 * b + 512 * qb) // RC
                                loc0 = 2048 * b + 512 * qb - RC * j2
                                nc.sync.dma_start(
                                    a2a_dst[65 * j2:65 * j2 + 65,
                                            loc0:loc0 + 512],
                                    ot65[:])
                        nc.gpsimd.collective_compute(
                            "AllToAll", ALU.bypass,
                            replica_groups=[list(range(NCORES))],
                            ins=[(a2a_inA if h == 0 else a2a_inB)[:].opt()],
                            outs=[(a2a_outA if h == 0 else a2a_outB)[:].opt()])

            # ---------------- phase 7: out-proj + residual -> h
            with tc.tile_pool(name="hp", bufs=1) as hp:
                h_t = [hp.tile([128, DIM], F32, tag=f"h{rt}", name=f"h{rt}")
                       for rt in range(8)]
                with (
                    tc.tile_pool(name="ps7", bufs=1, space="PSUM") as ps7,
                    tc.tile_pool(name="p7", bufs=1) as p7,
                    tc.tile_pool(name="p7w", bufs=6) as p7w,
                ):
                    ow_t = [[None, None] for _ in range(8)]
                    for nb in range(2):
                        for kc in range(8):
                            w = p7w.tile([128, 512], BF16, tag="ow", name="ow",
                                         bufs=16)
                            nc.sync.dma_start(
                                w[:], outw_in.ap()[128 * kc:128 * (kc + 1),
                                                   512 * nb:512 * (nb + 1)])
                            ow_t[kc][nb] = w
                    oT = []
                    for kc in range(8):
                        t = p7.tile([128, RC], BF16, tag=f"ot{kc}", name=f"oT{kc}")
                        nc.sync.dma_start(t[0:64, :],
                                          a2a_outA[65 * kc:65 * kc + 64, :])
                        nc.sync.dma_start(t[64:128, :],
                                          a2a_outB[65 * kc:65 * kc + 64, :])
                        oT.append(t)
                    rs_sb = p7.tile([16, RC], BF16, tag="rs", name="rs_sb")
                    for kc in range(8):
                        nc.sync.dma_start(
                            rs_sb[2 * kc:2 * kc + 1, :],
                            a2a_outA[65 * kc + 64:65 * kc + 65, :])
                        nc.sync.dma_start(
                            rs_sb[2 * kc + 1:2 * kc + 2, :],
                            a2a_outB[65 * kc + 64:65 * kc + 65, :])
                    rs_rec = p7.tile([16, RC], F32, tag="rsr", name="rs_rec")
                    nc.vector.reciprocal(rs_rec[:], rs_sb[:])
                    rs_rb = p7.tile([16, RC], BF16, tag="rsb", name="rs_rb")
                    nc.vector.tensor_copy(rs_rb[:], rs_rec[:])
                    for kc in range(8):
                        bc = p7w.tile([128, RC], BF16, tag="bc", name="bc",
                                      bufs=3)
                        for hh in range(2):
                            r1 = p7w.tile([1, RC], BF16, tag="r1b", name="r1b",
                                          bufs=3)
                            nc.sync.dma_start(
                                r1[:], rs_rb[2 * kc + hh:2 * kc + hh + 1, :])
                            if hh == 0:
                                nc.gpsimd.partition_broadcast(bc[0:64, :], r1[:])
                            else:
                                tb = p7w.tile([64, RC], BF16, tag="tb",
                                              name="tb", bufs=3)
                                nc.gpsimd.partition_broadcast(tb[:], r1[:])
                                nc.sync.dma_start(bc[64:128, :], tb[:])
                        nc.vector.tensor_mul(oT[kc][:], oT[kc][:], bc[:])
                    for nb in range(2):
                        pss = [ps7.tile([128, 512], F32, tag=f"mm{rt % 4}",
                                        name="psmm", bufs=2) for rt in range(8)]
                        for kc in range(8):
                            w = ow_t[kc][nb]
                            for rt in range(8):
                                nc.tensor.matmul(pss[rt][:],
                                                 oT[kc][:, 128 * rt:128 * (rt + 1)],
                                                 w[:], start=(kc == 0),
                                                 stop=(kc == 7))
                        for rt in range(8):
                            nc.vector.tensor_add(
                                h_t[rt][:, 512 * nb:512 * (nb + 1)], pss[rt][:],
                                x_t[rt][:, 512 * nb:512 * (nb + 1)])

                # ------------ phase 8: FFN (row-local), two halves of 512 rows
                with (
                    tc.tile_pool(name="ps8", bufs=1, space="PSUM") as ps8,
                    tc.tile_pool(name="ps8t", bufs=2, space="PSUM") as ps8t,
                    tc.tile_pool(name="p8", bufs=1) as p8,
                    tc.tile_pool(name="p8w", bufs=6) as p8w,
                    tc.tile_pool(name="p8s", bufs=3) as p8s,
                ):
                    for half in range(2):
                        # rms-norm h -> fn (bf16) -> transpose -> fnT
                        fnT = [p8.tile([128, 512], BF16, tag=f"fnT{fc}",
                                       name=f"fnT{fc}") for fc in range(8)]
                        for rt2 in range(4):
                            rt = 4 * half + rt2
                            rstd = _rms_rstd(nc, p8s, stats, h_t[rt], RMS_EPS)
                            fn = p8s.tile([128, DIM], BF16, tag="fn", name="fn")
                            nc.scalar.activation(fn[:], h_t[rt][:], AF.Copy,
                                                 bias=0.0, scale=rstd[:])
                            for fc in range(8):
                                ps = ps8t.tile([128, 128], BF16, tag="tr", name="tr")
                                nc.tensor.transpose(
                                    ps[:], fn[:, 128 * fc:128 * (fc + 1)], eye[:])
                                nc.vector.tensor_copy(
                                    fnT[fc][:, 128 * rt2:128 * (rt2 + 1)], ps[:])
                        # lin1 + GELU -> g [4 x 8 tiles of [128,512] bf16]
                        g_t = [[p8.tile([128, 512], BF16, tag=f"g{rt2}_{hb}",
                                        name=f"g{rt2}_{hb}")
                                for hb in range(8)] for rt2 in range(4)]
                        for hb in range(8):
                            pss = [ps8.tile([128, 512], F32, tag=f"mm{rt2}",
                                            name="psmm", bufs=1)
                                   for rt2 in range(4)]
                            for fc in range(8):
                                w = p8w.tile([128, 512], BF16, tag="l1w", name="l1w")
                                nc.sync.dma_start(
                                    w[:], l1w_in.ap()[128 * fc:128 * (fc + 1),
                                                      512 * hb:512 * (hb + 1)])
                                for rt2 in range(4):
                                    nc.tensor.matmul(
                                        pss[rt2][:],
                                        fnT[fc][:, 128 * rt2:128 * (rt2 + 1)],
                                        w[:], start=(fc == 0), stop=(fc == 7))
                            for rt2 in range(4):
                                nc.scalar.activation(g_t[rt2][hb][:], pss[rt2][:],
                                                     AF.Gelu)
                        # LayerNorm stats over hid (4096) per row
                        ab = []
                        for rt2 in range(4):
                            st = stats.tile([128, 8, 6], F32, tag="lnst",
                                            name="lnst")
                            for hb in range(8):
                                nc.vector.bn_stats(st[:, hb, :], g_t[rt2][hb][:])
                            mv = stats.tile([128, 2], F32, tag="lnmv", name="lnmv")
                            nc.vector.bn_aggr(mv[:], st[:])
                            std = stats.tile([128, 1], F32, tag="lnsd", name="lnsd")
                            nc.scalar.activation(std[:], mv[:, 1:2], AF.Sqrt,
                                                 bias=LN_EPS, scale=1.0)
                            rstd = stats.tile([128, 1], F32, tag="lnrs",
                                              name="lnrs")
                            nc.vector.reciprocal(rstd[:], std[:])
                            nmr = stats.tile([128, 1], F32, tag="lnnm", name="lnnm")
                            nc.vector.tensor_scalar(nmr[:], rstd[:], mv[:, 0:1],
                                                    -1.0, ALU.mult, ALU.mult)
                            ab.append((rstd, nmr))
                        # normalize + transpose -> gnT [32 tiles of [128,512]]
                        gnT = [p8.tile([128, 512], BF16, tag=f"gnT{hc}",
                                       name=f"gnT{hc}") for hc in range(32)]
                        for rt2 in range(4):
                            rstd, nmr = ab[rt2]
                            for hb in range(8):
                                gn = p8s.tile([128, 512], BF16, tag="gn", name="gn")
                                nc.vector.tensor_scalar(gn[:], g_t[rt2][hb][:],
                                                        rstd[:], nmr[:],
                                                        ALU.mult, ALU.add)
                                for j in range(4):
                                    ps = ps8t.tile([128, 128], BF16, tag="tr",
                                                   name="tr")
                                    nc.tensor.transpose(
                                        ps[:], gn[:, 128 * j:128 * (j + 1)], eye[:])
                                    nc.vector.tensor_copy(
                                        gnT[4 * hb + j][:, 128 * rt2:128 * (rt2 + 1)],
                                        ps[:])
                        # lin2 + residual -> y
                        for nb in range(2):
                            pss = [ps8.tile([128, 512], F32, tag=f"mm{rt2}",
                                            name="psmm", bufs=1)
                                   for rt2 in range(4)]
                            for hc in range(32):
                                w = p8w.tile([128, 512], BF16, tag="l2w", name="l2w")
                                nc.sync.dma_start(
                                    w[:], l2w_in.ap()[128 * hc:128 * (hc + 1),
                                                      512 * nb:512 * (nb + 1)])
                                for rt2 in range(4):
                                    nc.tensor.matmul(
                                        pss[rt2][:],
                                        gnT[hc][:, 128 * rt2:128 * (rt2 + 1)],
                                        w[:], start=(hc == 0), stop=(hc == 31))
                            for rt2 in range(4):
                                rt = 4 * half + rt2
                                yt = p8s.tile([128, 512], F32, tag="yt", name="yt")
                                nc.vector.tensor_add(
                                    yt[:], pss[rt2][:],
                                    h_t[rt][:, 512 * nb:512 * (nb + 1)])
                                nc.sync.dma_start(
                                    y_out.ap()[128 * rt:128 * (rt + 1),
                                               512 * nb:512 * (nb + 1)],
                                    yt[:])


# ----------------------------------------------------------------------------
# entry point
# ----------------------------------------------------------------------------

def kernel(x, mask, attn_scale, wqkv_w, wqkv_b, out_w, out_b,
           ffn_scale, lin1_w, lin1_b, ln_g, ln_b, lin2_w, lin2_b):
    x = np.asarray(x, np.float32)
    mask = np.asarray(mask, np.float32)

    lin2_b_eff = (np.asarray(lin2_b, np.float32)
                  + np.asarray(ln_b, np.float32) @ np.asarray(lin2_w, np.float32))
    if np.any(wqkv_b) or np.any(out_b) or np.any(lin1_b) or np.any(lin2_b_eff):
        return _numpy_fallback(x, mask, attn_scale, wqkv_w, wqkv_b, out_w, out_b,
                               ffn_scale, lin1_w, lin1_b, ln_g, ln_b, lin2_w,
                               lin2_b)

    actions, pmask_np = _classify_mask(mask)
    for qb in range(4):
        if all(actions[(kt, qb)] == "skip" for kt in range(16)):
            return _numpy_fallback(x, mask, attn_scale, wqkv_w, wqkv_b, out_w,
                                   out_b, ffn_scale, lin1_w, lin1_b, ln_g, ln_b,
                                   lin2_w, lin2_b)

    mask_sig = tuple(sorted((k, str(v)) for k, v in actions.items()))
    key = (mask_sig, pmask_np.shape[0])
    if key not in _PROGRAM_CACHE:
        _PROGRAM_CACHE[key] = _build_program(actions, pmask_np.shape[0])
    nc = _PROGRAM_CACHE[key]

    asc = np.asarray(attn_scale, np.float32)
    wqkv_eff = asc[:, None] * np.asarray(wqkv_w, np.float32)
    wq, wk, wv = (wqkv_eff[:, :DIM], wqkv_eff[:, DIM:2 * DIM],
                  wqkv_eff[:, 2 * DIM:])
    out_w_bf = _bf16(out_w)
    l1_bf = _bf16(np.asarray(ffn_scale, np.float32)[:, None]
                  * np.asarray(lin1_w, np.float32))
    l2_bf = _bf16(np.asarray(lin2_w, np.float32)
                  * np.asarray(ln_g, np.float32)[:, None])
    cosT, sinT = _rope_tables()
    eye = np.eye(128, dtype=ml_dtypes.bfloat16)

    x2 = np.ascontiguousarray(x.reshape(R, DIM))
    in_maps = []
    for c in range(NCORES):
        hA, hB = 2 * c, 2 * c + 1
        qA, qB = wq[:, 64 * hA:64 * hA + 64], wq[:, 64 * hB:64 * hB + 64]
        kA, kB = wk[:, 64 * hA:64 * hA + 64], wk[:, 64 * hB:64 * hB + 64]
        # interleaved for full-width rope: [qx1A qx1B kx1A kx1B | x2... | vA vB]
        sl = np.concatenate([qA[:, :32], qB[:, :32], kA[:, :32], kB[:, :32],
                             qA[:, 32:], qB[:, 32:], kA[:, 32:], kB[:, 32:],
                             wv[:, 128 * c:128 * (c + 1)]], axis=1)
        in_maps.append(dict(
            x_own=np.ascontiguousarray(x2[RC * c:RC * (c + 1)]),
            wqkv_sl=_bf16(sl),
            out_w=out_w_bf,
            lin1_w=l1_bf,
            lin2_w=l2_bf,
            cosT=cosT,
            sinT=sinT,
            pmask=pmask_np,
            eye=eye,
        ))

    global _LAST_IN_MAPS
    _LAST_IN_MAPS = in_maps
    res = run_bass_kernel_spmd(nc, in_maps, core_ids=list(range(NCORES)))
    y = np.concatenate([res.results[c]["y_own"] for c in range(NCORES)], axis=0)
    return y.reshape(B, L, DIM).astype(np.float32)


# revision 15
# speedup vs baseline: 1.3272x; 1.1653x over previous
"""Trainium2 Bass kernel for nn_Block_30313879175568 (dense transformer block).

Sharding: head-parallel attention (2 heads/core on 8 cores) + row-parallel
FFN/out-proj (1024 rows/core). Collectives: AllGather of rms-normed
activations (bf16, transposed layout), AllToAll of attention outputs
(heads -> rows). All matmul operands bf16 (fp32 PSUM accumulation); vector
math fp32.

Self-contained: imports only installed packages (concourse et al.) + numpy.
"""

import numpy as np
import ml_dtypes

import concourse.bass as bass  # noqa: F401
import concourse.mybir as mybir
import concourse.tile as tile
from concourse import bacc
from concourse.bass_utils import run_bass_kernel_spmd

BF16 = mybir.dt.bfloat16
F32 = mybir.dt.float32
AF = mybir.ActivationFunctionType
ALU = mybir.AluOpType

B, L, DIM, H, HID = 4, 2048, 1024, 16, 4096
HEAD_DIM = 64
NCORES = 8
R = B * L              # 8192 global rows
RC = R // NCORES       # 1024 rows per core
HPC = H // NCORES      # 2 heads per core
RMS_EPS = 1e-6
LN_EPS = 1e-5
VCOL = 2 * (HEAD_DIM + 1)   # 130: V cols per (batch,ktile) block incl ones
SH = 2 * HEAD_DIM + 2        # 130: a2a shard rows: 2x64 o^T dims + 2 rowsum rows

_PROGRAM_CACHE = {}
_LAST_IN_MAPS = None


# ----------------------------------------------------------------------------
# host-side helpers
# ----------------------------------------------------------------------------

def _bf16(a):
    return np.asarray(a, dtype=np.float32).astype(ml_dtypes.bfloat16)


def _rope_tables():
    half = HEAD_DIM // 2
    inv_freq = 10000.0 ** (-np.arange(0, half, dtype=np.float32) * 2.0 / HEAD_DIM)
    pos = np.arange(L, dtype=np.float32)
    theta = pos[:, None] * inv_freq[None, :]          # [L, 32]
    cos = np.cos(theta).T.astype(np.float32)          # [32, L]
    sin = np.sin(theta).T.astype(np.float32)
    return (np.tile(cos, (4, 1)).copy(), np.tile(sin, (4, 1)).copy())  # [128, L]


def _classify_mask(mask):
    """Split mask^T [k, q] into (16 ktile x 4 qblock) blocks.

    Returns (actions, pmask_np): actions[(kt, qb)] is 'skip' | 'noop' |
    ('mul', idx); pmask_np is [NU, 128, 512] bf16 of exp(mask^T block).
    """
    maskT = np.asarray(mask, dtype=np.float32).T
    actions = {}
    uniq = {}
    tiles = []
    for qb in range(4):
        for kt in range(16):
            blk = maskT[128 * kt:128 * (kt + 1), 512 * qb:512 * (qb + 1)]
            if np.all(blk <= -30.0):
                actions[(kt, qb)] = "skip"
            elif np.all(blk == 0.0):
                actions[(kt, qb)] = "noop"
            else:
                pm = _bf16(np.exp(blk.astype(np.float64)))
                key = pm.tobytes()
                if key not in uniq:
                    uniq[key] = len(tiles)
                    tiles.append(pm)
                actions[(kt, qb)] = ("mul", uniq[key])
    if not tiles:
        tiles = [np.zeros((128, 512), dtype=ml_dtypes.bfloat16)]
    pmask_np = np.stack(tiles, axis=0)
    return actions, pmask_np


def _numpy_fallback(x, mask, attn_scale, wqkv_w, wqkv_b, out_w, out_b,
                    ffn_scale, lin1_w, lin1_b, ln_g, ln_b, lin2_w, lin2_b):
    """Correct (slow) host fallback for configurations the device program
    doesn't support (nonzero biases / fully-masked rows)."""
    from scipy.special import erf

    def rms(t, scale):
        return t / np.sqrt(np.mean(t * t, axis=-1, keepdims=True) + RMS_EPS) * scale

    x = np.asarray(x, np.float64)
    xn = rms(x, attn_scale)
    qkv = xn @ np.asarray(wqkv_w, np.float64) + wqkv_b
    q, k, v = np.split(qkv, 3, axis=-1)
    th = lambda t: t.reshape(B, L, H, HEAD_DIM).transpose(0, 2, 1, 3)
    q, k, v = th(q), th(k), th(v)

    half = HEAD_DIM // 2
    inv_freq = 10000.0 ** (-np.arange(0, half) * 2.0 / HEAD_DIM)
    theta = np.arange(L)[:, None] * inv_freq[None, :]
    cos, sin = np.cos(theta), np.sin(theta)

    def rope(t):
        x1, x2 = t[..., :half], t[..., half:]
        return np.concatenate([x1 * cos - x2 * sin, x1 * sin + x2 * cos], axis=-1)

    q, k = rope(q), rope(k)
    s = np.einsum("bhqd,bhkd->bhqk", q / np.sqrt(HEAD_DIM), k) + np.asarray(mask, np.float64)
    s = s - s.max(axis=-1, keepdims=True)
    p = np.exp(s)
    p /= p.sum(axis=-1, keepdims=True)
    o = np.einsum("bhqk,bhkd->bhqd", p, v)
    o = o.transpose(0, 2, 1, 3).reshape(B, L, DIM)
    h = x + o @ np.asarray(out_w, np.float64) + out_b
    f = rms(h, ffn_scale)
    f = f @ np.asarray(lin1_w, np.float64) + lin1_b
    f = 0.5 * f * (1.0 + erf(f / np.sqrt(2.0)))
    mu = f.mean(axis=-1, keepdims=True)
    var = f.var(axis=-1, keepdims=True)
    f = (f - mu) / np.sqrt(var + LN_EPS) * ln_g + ln_b
    out = h + f @ np.asarray(lin2_w, np.float64) + lin2_b
    return out.astype(np.float32)


# ----------------------------------------------------------------------------
# device program
# ----------------------------------------------------------------------------

def _rms_rstd(nc, scratch, stats, t, eps):
    """1/sqrt(mean(t^2, free) + eps) for a [128, D] f32 tile, via ACT."""
    D = t.shape[1]
    sq = scratch.tile([128, D], BF16, tag="sq", name="sq")
    ssq = stats.tile([128, 1], F32, tag="ssq", name="ssq")
    nc.scalar.activation(sq[:], t[:], AF.Square, accum_out=ssq[:])
    std = stats.tile([128, 1], F32, tag="rmssd", name="rmssd")
    nc.scalar.activation(std[:], ssq[:], AF.Sqrt, bias=eps, scale=1.0 / D)
    rstd = stats.tile([128, 1], F32, tag="rmsrs", name="rmsrs")
    nc.vector.reciprocal(rstd[:], std[:])
    return rstd


def _register_const(nc, value, dtype=F32):
    t = nc.alloc_sbuf_tensor(f"const-{dtype.name}-{value}", [128, 1], dtype)
    nc.gpsimd.memset(t.ap(), value)
    nc.const_aps.aps[(dtype, value)] = t.ap()


def _build_program(actions, n_pmask):
    nc = bacc.Bacc("TRN2", target_bir_lowering=False, debug=False,
                   num_devices=NCORES)
    _register_const(nc, RMS_EPS)
    _register_const(nc, LN_EPS)
    nc.all_engine_barrier()

    x_in = nc.dram_tensor("x_own", [RC, DIM], F32, kind="ExternalInput")
    wqkv_in = nc.dram_tensor("wqkv_sl", [DIM, 3 * 128], BF16, kind="ExternalInput")
    outw_in = nc.dram_tensor("out_w", [DIM, DIM], BF16, kind="ExternalInput")
    l1w_in = nc.dram_tensor("lin1_w", [DIM, HID], BF16, kind="ExternalInput")
    l2w_in = nc.dram_tensor("lin2_w", [HID, DIM], BF16, kind="ExternalInput")
    cos_in = nc.dram_tensor("cosT", [128, L], F32, kind="ExternalInput")
    sin_in = nc.dram_tensor("sinT", [128, L], F32, kind="ExternalInput")
    pm_in = nc.dram_tensor("pmask", [n_pmask, 128, 512], BF16, kind="ExternalInput")
    eye_in = nc.dram_tensor("eye", [128, 128], BF16, kind="ExternalInput")
    y_out = nc.dram_tensor("y_own", [RC, DIM], F32, kind="ExternalOutput")

    with tile.TileContext(nc) as tc:
        _emit(nc, tc, x_in, wqkv_in, outw_in, l1w_in, l2w_in, cos_in, sin_in,
              pm_in, eye_in, y_out, actions, n_pmask)

    nc.compile()
    return nc


def _emit(nc, tc, x_in, wqkv_in, outw_in, l1w_in, l2w_in, cos_in, sin_in,
          pm_in, eye_in, y_out, actions, n_pmask):
    with (
        tc.tile_pool(name="dram", bufs=1, space="DRAM") as dram,
        tc.tile_pool(name="base", bufs=1) as base,
        tc.tile_pool(name="work", bufs=4) as work,
        tc.tile_pool(name="stats", bufs=4) as stats,
    ):
        eye = base.tile([128, 128], BF16)
        nc.sync.dma_start(eye[:], eye_in.ap())

        ag_in0 = dram.tile([DIM, RC // 2], BF16)
        ag_in1 = dram.tile([DIM, RC // 2], BF16)
        ag_out0 = dram.tile([NCORES * DIM, RC // 2], BF16, addr_space="Shared")
        ag_out1 = dram.tile([NCORES * DIM, RC // 2], BF16, addr_space="Shared")
        a2a_inA = dram.tile([NCORES * 65, RC], BF16)
        a2a_outA = dram.tile([NCORES * 65, RC], BF16)
        a2a_inB = dram.tile([NCORES * 65, RC], BF16)
        a2a_outB = dram.tile([NCORES * 65, RC], BF16)

        with tc.tile_pool(name="xp", bufs=1) as xp:
            x_t = [xp.tile([128, DIM], F32, tag=f"x{rt}", name=f"x{rt}")
                   for rt in range(8)]

            # ---------------- phase 1: load x, rms-norm, transpose -> ag_in
            with (
                tc.tile_pool(name="ps1", bufs=2, space="PSUM") as ps1,
                tc.tile_pool(name="p1", bufs=3) as p1,
            ):
                for rt in range(8):
                    nc.sync.dma_start(x_t[rt][:], x_in.ap()[128 * rt:128 * (rt + 1), :])
                    rstd = _rms_rstd(nc, p1, stats, x_t[rt], RMS_EPS)
                    xn = p1.tile([128, DIM], BF16, tag="xn", name="xn")
                    nc.scalar.activation(xn[:], x_t[rt][:], AF.Copy, bias=0.0,
                                         scale=rstd[:])
                    for fc in range(8):
                        ps = ps1.tile([128, 128], BF16, tag="tr", name="tr")
                        nc.tensor.transpose(ps[:], xn[:, 128 * fc:128 * (fc + 1)], eye[:])
                        ev = p1.tile([128, 128], BF16, tag="ev", name="ev")
                        nc.vector.tensor_copy(ev[:], ps[:])
                        agd = ag_in0 if rt < 4 else ag_in1
                        lrt = rt % 4
                        nc.sync.dma_start(
                            agd[128 * fc:128 * (fc + 1), 128 * lrt:128 * (lrt + 1)],
                            ev[:])
                    if rt == 3:
                        nc.gpsimd.collective_compute(
                            "AllGather", ALU.bypass,
                            replica_groups=[list(range(NCORES))],
                            ins=[ag_in0[:].opt()], outs=[ag_out0[:].opt()])
                    if rt == 7:
                        nc.gpsimd.collective_compute(
                            "AllGather", ALU.bypass,
                            replica_groups=[list(range(NCORES))],
                            ins=[ag_in1[:].opt()], outs=[ag_out1[:].opt()])

            with (
                tc.tile_pool(name="attn", bufs=1) as attn,
                tc.tile_pool(name="cst", bufs=1) as cst,
            ):
                cosT = cst.tile([128, L], F32)
                sinT = cst.tile([128, L], F32)
                nc.sync.dma_start(cosT[:], cos_in.ap())
                nc.sync.dma_start(sinT[:], sin_in.ap())
                pm_t = []
                for i in range(n_pmask):
                    t = cst.tile([128, 512], BF16, tag=f"pm{i}", name=f"pm{i}")
                    nc.sync.dma_start(t[:], pm_in.ap()[i, :, :])
                    pm_t.append(t)
                w_t = []
                for kc in range(8):
                    t = cst.tile([128, 3 * 128], BF16, tag=f"w{kc}", name=f"w{kc}")
                    nc.sync.dma_start(t[:], wqkv_in.ap()[128 * kc:128 * (kc + 1), :])
                    w_t.append(t)

                # QTA: head-A Q in rows 0-63, zeros in 64-127 (so S-matmuls
                # contract over all 128 partitions = full PE array = warm HAM).
                QTA = attn.tile([128, R], BF16)
                QTB = attn.tile([128, R], BF16)
                KT = attn.tile([128, R], BF16)
                VT = attn.tile([128, R], BF16)
                V_sb = attn.tile([128, 64 * VCOL], BF16)
                nc.vector.memset(V_sb[:], 1.0)
                nc.vector.memset(QTA[64:128, :], 0.0)
                nc.vector.memset(QTB[0:64, :], 0.0)

                # ------------ phase 3: QKV + RoPE
                with (
                    tc.tile_pool(name="ps3", bufs=2, space="PSUM") as ps3,
                    tc.tile_pool(name="p3", bufs=3) as p3,
                ):
                    for blk in [0, 2, 4, 6, 8, 10, 12, 14,
                                1, 3, 5, 7, 9, 11, 13, 15]:
                        cb, half = blk // 2, blk % 2
                        ag_src = ag_out0 if half == 0 else ag_out1
                        pos0 = 512 * (blk % 4)
                        g0 = 512 * blk
                        xnt = []
                        for kc in range(8):
                            t = p3.tile([128, 512], BF16, tag="xnt", name="xnt",
                                        bufs=12)
                            nc.sync.dma_start(
                                t[:],
                                ag_src[DIM * cb + 128 * kc:DIM * cb + 128 * (kc + 1),
                                       :])
                            xnt.append(t)
                        psX1 = ps3.tile([128, 512], F32, tag="q", name="psx1")
                        psX2 = ps3.tile([128, 512], F32, tag="k", name="psx2")
                        psV = ps3.tile([128, 512], F32, tag="v", name="psv")
                        for kc in range(8):
                            st, sp = kc == 0, kc == 7
                            nc.tensor.matmul(psX1[:], w_t[kc][:, 0:128], xnt[kc][:],
                                             start=st, stop=sp)
                            nc.tensor.matmul(psX2[:], w_t[kc][:, 128:256], xnt[kc][:],
                                             start=st, stop=sp)
                            nc.tensor.matmul(psV[:], w_t[kc][:, 256:384], xnt[kc][:],
                                             start=st, stop=sp)
                        # full-width rope: psX1 rows = [qx1A qx1B kx1A kx1B],
                        # psX2 rows = [qx2A qx2B kx2A kx2B]
                        cs = cosT[:, pos0:pos0 + 512]
                        sn = sinT[:, pos0:pos0 + 512]
                        t1 = work.tile([128, 512], F32, tag="r1", name="r1")
                        t2 = work.tile([128, 512], F32, tag="r2", name="r2")
                        o1 = p3.tile([128, 512], BF16, tag="o1", name="o1")
                        o2 = p3.tile([128, 512], BF16, tag="o2", name="o2")
                        nc.vector.tensor_mul(t1[:], psX1[:], cs)
                        nc.vector.tensor_mul(t2[:], psX2[:], sn)
                        nc.vector.tensor_sub(o1[:], t1[:], t2[:])
                        nc.vector.tensor_mul(t1[:], psX1[:], sn)
                        nc.vector.tensor_mul(t2[:], psX2[:], cs)
                        nc.vector.tensor_add(o2[:], t1[:], t2[:])
                        # reassemble: QTA rows 0-63 = head A, QTB rows 64-127
                        # = head B; KT = [headA d0-63, headB d0-63]
                        for src_t, d0, dst, dr in (
                            (o1, 0, QTA, 0), (o2, 0, QTA, 32),
                            (o1, 32, QTB, 64), (o2, 32, QTB, 96),
                            (o1, 64, KT, 0), (o2, 64, KT, 32),
                            (o1, 96, KT, 64), (o2, 96, KT, 96),
                        ):
                            nc.sync.dma_start(dst[dr:dr + 32, g0:g0 + 512],
                                              src_t[d0:d0 + 32, :])
                        nc.vector.tensor_copy(VT[:, g0:g0 + 512], psV[:])

                    # ------------ phase 4: V^T -> row-major V blocks (+ones)
                    for bt in range(64):
                        pst = ps3.tile([128, 128], BF16, tag="tr", name="tr")
                        nc.tensor.transpose(pst[:], VT[:, 128 * bt:128 * (bt + 1)],
                                            eye[:])
                        nc.vector.tensor_copy(V_sb[:, VCOL * bt:VCOL * bt + 64],
                                              pst[:, 0:64])
                        nc.vector.tensor_copy(V_sb[:, VCOL * bt + 65:VCOL * bt + 129],
                                              pst[:, 64:128])

                # ------------ phase 5: attention per (batch, head, qblock)
                with (
                    tc.tile_pool(name="ps5s", bufs=5, space="PSUM") as ps5s,
                    tc.tile_pool(name="ps5o", bufs=3, space="PSUM") as ps5o,
                    tc.tile_pool(name="p5", bufs=3) as p5,
                ):
                    LOOK = 3
                    for h in range(HPC):
                        hr = 64 * h
                        a2a_dst = a2a_inA if h == 0 else a2a_inB
                        for b in range(B):
                            for qb in range(4):
                                q0 = 2048 * b + 512 * qb
                                act = [(kt, actions[(kt, qb)]) for kt in range(16)
                                       if actions[(kt, qb)] != "skip"]
                                n = len(act)
                                psO = ps5o.tile([65, 512], F32, tag="o", name="pso")
                                pts = []
                                for i in range(n + LOOK):
                                    if i < n:
                                        kt, a = act[i]
                                        k0 = 2048 * b + 128 * kt
                                        psS = ps5s.tile([128, 512], F32, tag="s",
                                                        name="pss")
                                        QTh = QTA if h == 0 else QTB
                                        nc.tensor.matmul(
                                            psS[:], KT[:, k0:k0 + 128],
                                            QTh[:, q0:q0 + 512],
                                            start=True, stop=True)
                                        pt = p5.tile([128, 512], BF16, tag="pt",
                                                     name="pt", bufs=8)
                                        nc.scalar.activation(pt[:], psS[:], AF.Exp,
                                                             bias=0.0, scale=0.125)
                                        if a != "noop":
                                            nc.vector.tensor_mul(pt[:], pt[:],
                                                                 pm_t[a[1]][:])
                                        pts.append((kt, pt))
                                    j = i - LOOK
                                    if 0 <= j < n:
                                        kt, pt = pts[j]
                                        bt = 16 * b + kt
                                        nc.tensor.matmul(
                                            psO[:],
                                            V_sb[:, VCOL * bt + 65 * h:
                                                 VCOL * bt + 65 * h + 65],
                                            pt[:],
                                            start=(j == 0), stop=(j == n - 1))
                                ot65 = p5.tile([65, 512], BF16, tag="ot",
                                               name="ot65")
                                nc.vector.tensor_copy(ot65[:], psO[:, :])
                                j2 = (2048 * b + 512 * qb) // RC
                                loc0 = 2048 * b + 512 * qb - RC * j2
                                nc.sync.dma_start(
                                    a2a_dst[65 * j2:65 * j2 + 65,
                                            loc0:loc0 + 512],
                                    ot65[:])
                        nc.gpsimd.collective_compute(
                            "AllToAll", ALU.bypass,
                            replica_groups=[list(range(NCORES))],
                            ins=[(a2a_inA if h == 0 else a2a_inB)[:].opt()],
                            outs=[(a2a_outA if h == 0 else a2a_outB)[:].opt()])

            # ---------------- phase 7: out-proj + residual -> h
            with tc.tile_pool(name="hp", bufs=1) as hp:
                h_t = [hp.tile([128, DIM], F32, tag=f"h{rt}", name=f"h{rt}")
                       for rt in range(8)]
                with (
                    tc.tile_pool(name="ps7", bufs=1, space="PSUM") as ps7,
                    tc.tile_pool(name="p7", bufs=1) as p7,
                    tc.tile_pool(name="p7w", bufs=6) as p7w,
                ):
                    ow_t = [[None, None] for _ in range(8)]
                    for nb in range(2):
                        for kc in range(8):
                            w = p7w.tile([128, 512], BF16, tag="ow", name="ow",
                                         bufs=16)
                            nc.sync.dma_start(
                                w[:], outw_in.ap()[128 * kc:128 * (kc + 1),
                                                   512 * nb:512 * (nb + 1)])
                            ow_t[kc][nb] = w
                    oT = []
                    for kc in range(8):
                        t = p7.tile([128, RC], BF16, tag=f"ot{kc}", name=f"oT{kc}")
                        nc.sync.dma_start(t[0:64, :],
                                          a2a_outA[65 * kc:65 * kc + 64, :])
                        nc.sync.dma_start(t[64:128, :],
                                          a2a_outB[65 * kc:65 * kc + 64, :])
                        oT.append(t)
                    rs_sb = p7.tile([16, RC], BF16, tag="rs", name="rs_sb")
                    for kc in range(8):
                        nc.sync.dma_start(
                            rs_sb[2 * kc:2 * kc + 1, :],
                            a2a_outA[65 * kc + 64:65 * kc + 65, :])
                        nc.sync.dma_start(
                            rs_sb[2 * kc + 1:2 * kc + 2, :],
                            a2a_outB[65 * kc + 64:65 * kc + 65, :])
                    rs_rec = p7.tile([16, RC], F32, tag="rsr", name="rs_rec")
                    nc.vector.reciprocal(rs_rec[:], rs_sb[:])
                    rs_rb = p7.tile([16, RC], BF16, tag="rsb", name="rs_rb")
                    nc.vector.tensor_copy(rs_rb[:], rs_rec[:])
                    for kc in range(8):
                        bc = p7w.tile([128, RC], BF16, tag="bc", name="bc",
                                      bufs=3)
                        for hh in range(2):
                            r1 = p7w.tile([1, RC], BF16, tag="r1b", name="r1b",
                                          bufs=3)
                            nc.sync.dma_start(
                                r1[:], rs_rb[2 * kc + hh:2 * kc + hh + 1, :])
                            if hh == 0:
                                nc.gpsimd.partition_broadcast(bc[0:64, :], r1[:])
                            else:
                                tb = p7w.tile([64, RC], BF16, tag="tb",
                                              name="tb", bufs=3)
                                nc.gpsimd.partition_broadcast(tb[:], r1[:])
                                nc.sync.dma_start(bc[64:128, :], tb[:])
                        nc.vector.tensor_mul(oT[kc][:], oT[kc][:], bc[:])
                    for nb in range(2):
                        pss = [ps7.tile([128, 512], F32, tag=f"mm{rt % 4}",
                                        name="psmm", bufs=2) for rt in range(8)]
                        for kc in range(8):
                            w = ow_t[kc][nb]
                            for rt in range(8):
                                nc.tensor.matmul(pss[rt][:],
                                                 oT[kc][:, 128 * rt:128 * (rt + 1)],
                                                 w[:], start=(kc == 0),
                                                 stop=(kc == 7))
                        for rt in range(8):
                            nc.vector.tensor_add(
                                h_t[rt][:, 512 * nb:512 * (nb + 1)], pss[rt][:],
                                x_t[rt][:, 512 * nb:512 * (nb + 1)])

                # ------------ phase 8: FFN (row-local), two halves of 512 rows
                with (
                    tc.tile_pool(name="ps8", bufs=1, space="PSUM") as ps8,
                    tc.tile_pool(name="ps8t", bufs=2, space="PSUM") as ps8t,
                    tc.tile_pool(name="p8", bufs=1) as p8,
                    tc.tile_pool(name="p8w", bufs=6) as p8w,
                    tc.tile_pool(name="p8s", bufs=3) as p8s,
                ):
                    for half in range(2):
                        # rms-norm h -> fn (bf16) -> transpose -> fnT
                        fnT = [p8.tile([128, 512], BF16, tag=f"fnT{fc}",
                                       name=f"fnT{fc}") for fc in range(8)]
                        for rt2 in range(4):
                            rt = 4 * half + rt2
                            rstd = _rms_rstd(nc, p8s, stats, h_t[rt], RMS_EPS)
                            fn = p8s.tile([128, DIM], BF16, tag="fn", name="fn")
                            nc.scalar.activation(fn[:], h_t[rt][:], AF.Copy,
                                                 bias=0.0, scale=rstd[:])
                            for fc in range(8):
                                ps = ps8t.tile([128, 128], BF16, tag="tr", name="tr")
                                nc.tensor.transpose(
                                    ps[:], fn[:, 128 * fc:128 * (fc + 1)], eye[:])
                                nc.vector.tensor_copy(
                                    fnT[fc][:, 128 * rt2:128 * (rt2 + 1)], ps[:])
                        # lin1 + GELU -> g [4 x 8 tiles of [128,512] bf16]
                        g_t = [[p8.tile([128, 512], BF16, tag=f"g{rt2}_{hb}",
                                        name=f"g{rt2}_{hb}")
                                for hb in range(8)] for rt2 in range(4)]
                        for hb in range(8):
                            pss = [ps8.tile([128, 512], F32, tag=f"mm{rt2}",
                                            name="psmm", bufs=1)
                                   for rt2 in range(4)]
                            for fc in range(8):
                                w = p8w.tile([128, 512], BF16, tag="l1w", name="l1w")
                                nc.sync.dma_start(
                                    w[:], l1w_in.ap()[128 * fc:128 * (fc + 1),
                                                      512 * hb:512 * (hb + 1)])
                                for rt2 in range(4):
                                    nc.tensor.matmul(
                                        pss[rt2][:],
                                        fnT[fc][:, 128 * rt2:128 * (rt2 + 1)],
                                        w[:], start=(fc == 0), stop=(fc == 7))
                            for rt2 in range(4):
                                nc.scalar.activation(g_t[rt2][hb][:], pss[rt2][:],
                                                     AF.Gelu)
                        # LayerNorm stats over hid (4096) per row
                        ab = []
                        for rt2 in range(4):
                            st = stats.tile([128, 8, 6], F32, tag="lnst",
                                            name="lnst")
                            for hb in range(8):
                                nc.vector.bn_stats(st[:, hb, :], g_t[rt2][hb][:])
                            mv = stats.tile([128, 2], F32, tag="lnmv", name="lnmv")
                            nc.vector.bn_aggr(mv[:], st[:])
                            std = stats.tile([128, 1], F32, tag="lnsd", name="lnsd")
                            nc.scalar.activation(std[:], mv[:, 1:2], AF.Sqrt,
                                                 bias=LN_EPS, scale=1.0)
                            rstd = stats.tile([128, 1], F32, tag="lnrs",
                                              name="lnrs")
                            nc.vector.reciprocal(rstd[:], std[:])
                            nmr = stats.tile([128, 1], F32, tag="lnnm", name="lnnm")
                            nc.vector.tensor_scalar(nmr[:], rstd[:], mv[:, 0:1],
                                                    -1.0, ALU.mult, ALU.mult)
                            ab.append((rstd, nmr))
                        # normalize + transpose -> gnT [32 tiles of [128,512]]
                        gnT = [p8.tile([128, 512], BF16, tag=f"gnT{hc}",
                                       name=f"gnT{hc}") for hc in range(32)]
                        for rt2 in range(4):
                            rstd, nmr = ab[rt2]
                            for hb in range(8):
                                gn = p8s.tile([128, 512], BF16, tag="gn", name="gn")
                                nc.vector.tensor_scalar(gn[:], g_t[rt2][hb][:],
                                                        rstd[:], nmr[:],
                                                        ALU.mult, ALU.add)
                                for j in range(4):
                                    ps = ps8t.tile([128, 128], BF16, tag="tr",
                                                   name="tr")
                                    nc.tensor.transpose(
                                        ps[:], gn[:, 128 * j:128 * (j + 1)], eye[:])
                                    nc.vector.tensor_copy(
                                        gnT[4 * hb + j][:, 128 * rt2:128 * (rt2 + 1)],
                                        ps[:])
                        # lin2 + residual -> y
                        for nb in range(2):
                            pss = [ps8.tile([128, 512], F32, tag=f"mm{rt2}",
                                            name="psmm", bufs=1)
                                   for rt2 in range(4)]
                            for hc in range(32):
                                w = p8w.tile([128, 512], BF16, tag="l2w", name="l2w")
                                nc.sync.dma_start(
                                    w[:], l2w_in.ap()[128 * hc:128 * (hc + 1),
                                                      512 * nb:512 * (nb + 1)])
                                for rt2 in range(4):
                                    nc.tensor.matmul(
                                        pss[rt2][:],
                                        gnT[hc][:, 128 * rt2:128 * (rt2 + 1)],
                                        w[:], start=(hc == 0), stop=(hc == 31))
                            for rt2 in range(4):
                                rt = 4 * half + rt2
                                yt = p8s.tile([128, 512], F32, tag="yt", name="yt")
                                nc.vector.tensor_add(
                                    yt[:], pss[rt2][:],
                                    h_t[rt][:, 512 * nb:512 * (nb + 1)])
                                nc.sync.dma_start(
                                    y_out.ap()[128 * rt:128 * (rt + 1),
                                               512 * nb:512 * (nb + 1)],
                                    yt[:])


# ----------------------------------------------------------------------------
# entry point
# ----------------------------------------------------------------------------

def kernel(x, mask, attn_scale, wqkv_w, wqkv_b, out_w, out_b,
           ffn_scale, lin1_w, lin1_b, ln_g, ln_b, lin2_w, lin2_b):
    x = np.asarray(x, np.float32)
    mask = np.asarray(mask, np.float32)

    lin2_b_eff = (np.asarray(lin2_b, np.float32)
                  + np.asarray(ln_b, np.float32) @ np.asarray(lin2_w, np.float32))
    if np.any(wqkv_b) or np.any(out_b) or np.any(lin1_b) or np.any(lin2_b_eff):
        return _numpy_fallback(x, mask, attn_scale, wqkv_w, wqkv_b, out_w, out_b,
                               ffn_scale, lin1_w, lin1_b, ln_g, ln_b, lin2_w,
                               lin2_b)

    actions, pmask_np = _classify_mask(mask)
    for qb in range(4):
        if all(actions[(kt, qb)] == "skip" for kt in range(16)):
            return _numpy_fallback(x, mask, attn_scale, wqkv_w, wqkv_b, out_w,
                                   out_b, ffn_scale, lin1_w, lin1_b, ln_g, ln_b,
                                   lin2_w, lin2_b)

    mask_sig = tuple(sorted((k, str(v)) for k, v in actions.items()))
    key = (mask_sig, pmask_np.shape[0])
    if key not in _PROGRAM_CACHE:
        _PROGRAM_CACHE[key] = _build_program(actions, pmask_np.shape[0])
    nc = _PROGRAM_CACHE[key]

    asc = np.asarray(attn_scale, np.float32)
    wqkv_eff = asc[:, None] * np.asarray(wqkv_w, np.float32)
    wq, wk, wv = (wqkv_eff[:, :DIM], wqkv_eff[:, DIM:2 * DIM],
                  wqkv_eff[:, 2 * DIM:])
    out_w_bf = _bf16(out_w)
    l1_bf = _bf16(np.asarray(ffn_scale, np.float32)[:, None]
                  * np.asarray(lin1_w, np.float32))
    l2_bf = _bf16(np.asarray(lin2_w, np.float32)
                  * np.asarray(ln_g, np.float32)[:, None])
    cosT, sinT = _rope_tables()
    eye = np.eye(128, dtype=ml_dtypes.bfloat16)

    x2 = np.ascontiguousarray(x.reshape(R, DIM))
    in_maps = []
    for c in range(NCORES):
        hA, hB = 2 * c, 2 * c + 1
        qA, qB = wq[:, 64 * hA:64 * hA + 64], wq[:, 64 * hB:64 * hB + 64]
        kA, kB = wk[:, 64 * hA:64 * hA + 64], wk[:, 64 * hB:64 * hB + 64]
        # interleaved for full-width rope: [qx1A qx1B kx1A kx1B | x2... | vA vB]
        sl = np.concatenate([qA[:, :32], qB[:, :32], kA[:, :32], kB[:, :32],
                             qA[:, 32:], qB[:, 32:], kA[:, 32:], kB[:, 32:],
                             wv[:, 128 * c:128 * (c + 1)]], axis=1)
        in_maps.append(dict(
            x_own=np.ascontiguousarray(x2[RC * c:RC * (c + 1)]),
            wqkv_sl=_bf16(sl),
            out_w=out_w_bf,
            lin1_w=l1_bf,
            lin2_w=l2_bf,
            cosT=cosT,
            sinT=sinT,
            pmask=pmask_np,
            eye=eye,
        ))

    global _LAST_IN_MAPS
    _LAST_IN_MAPS = in_maps
    res = run_bass_kernel_spmd(nc, in_maps, core_ids=list(range(NCORES)))
    y = np.concatenate([res.results[c]["y_own"] for c in range(NCORES)], axis=0)
    return y.reshape(B, L, DIM).astype(np.float32)


# revision 18
# speedup vs baseline: 1.4192x; 1.0693x over previous
"""Trainium2 Bass kernel for nn_Block_30313879175568 (dense transformer block).

Sharding: head-parallel attention (2 heads/core on 8 cores) + row-parallel
FFN/out-proj (1024 rows/core). Collectives: AllGather of rms-normed
activations (bf16, transposed layout), AllToAll of attention outputs
(heads -> rows). All matmul operands bf16 (fp32 PSUM accumulation); vector
math fp32.

Self-contained: imports only installed packages (concourse et al.) + numpy.
"""

import numpy as np
import ml_dtypes

import concourse.bass as bass  # noqa: F401
import concourse.mybir as mybir
import concourse.tile as tile
from concourse import bacc
from concourse.bass_utils import run_bass_kernel_spmd

BF16 = mybir.dt.bfloat16
F32 = mybir.dt.float32
AF = mybir.ActivationFunctionType
ALU = mybir.AluOpType

B, L, DIM, H, HID = 4, 2048, 1024, 16, 4096
HEAD_DIM = 64
NCORES = 8
R = B * L              # 8192 global rows
RC = R // NCORES       # 1024 rows per core
HPC = H // NCORES      # 2 heads per core
RMS_EPS = 1e-6
LN_EPS = 1e-5
VCOL = 2 * (HEAD_DIM + 1)   # 130: V cols per (batch,ktile) block incl ones
SH = 2 * HEAD_DIM + 2        # 130: a2a shard rows: 2x64 o^T dims + 2 rowsum rows

_PROGRAM_CACHE = {}
_LAST_IN_MAPS = None


# ----------------------------------------------------------------------------
# host-side helpers
# ----------------------------------------------------------------------------

def _bf16(a):
    return np.asarray(a, dtype=np.float32).astype(ml_dtypes.bfloat16)


def _rope_tables():
    half = HEAD_DIM // 2
    inv_freq = 10000.0 ** (-np.arange(0, half, dtype=np.float32) * 2.0 / HEAD_DIM)
    pos = np.arange(L, dtype=np.float32)
    theta = pos[:, None] * inv_freq[None, :]          # [L, 32]
    cos = np.cos(theta).T.astype(np.float32)          # [32, L]
    sin = np.sin(theta).T.astype(np.float32)
    return (np.tile(cos, (4, 1)).copy(), np.tile(sin, (4, 1)).copy())  # [128, L]


def _classify_mask(mask):
    """Split mask^T [k, q] into (16 ktile x 4 qblock) blocks.

    Returns (actions, pmask_np): actions[(kt, qb)] is 'skip' | 'noop' |
    ('mul', idx); pmask_np is [NU, 128, 512] bf16 of exp(mask^T block).
    """
    maskT = np.asarray(mask, dtype=np.float32).T
    actions = {}
    uniq = {}
    tiles = []
    for qb in range(4):
        for kt in range(16):
            blk = maskT[128 * kt:128 * (kt + 1), 512 * qb:512 * (qb + 1)]
            if np.all(blk <= -30.0):
                actions[(kt, qb)] = "skip"
            elif np.all(blk == 0.0):
                actions[(kt, qb)] = "noop"
            else:
                pm = _bf16(np.exp(blk.astype(np.float64)))
                key = pm.tobytes()
                if key not in uniq:
                    uniq[key] = len(tiles)
                    tiles.append(pm)
                actions[(kt, qb)] = ("mul", uniq[key])
    if not tiles:
        tiles = [np.zeros((128, 512), dtype=ml_dtypes.bfloat16)]
    pmask_np = np.stack(tiles, axis=0)
    return actions, pmask_np


def _numpy_fallback(x, mask, attn_scale, wqkv_w, wqkv_b, out_w, out_b,
                    ffn_scale, lin1_w, lin1_b, ln_g, ln_b, lin2_w, lin2_b):
    """Correct (slow) host fallback for configurations the device program
    doesn't support (nonzero biases / fully-masked rows)."""
    from scipy.special import erf

    def rms(t, scale):
        return t / np.sqrt(np.mean(t * t, axis=-1, keepdims=True) + RMS_EPS) * scale

    x = np.asarray(x, np.float64)
    xn = rms(x, attn_scale)
    qkv = xn @ np.asarray(wqkv_w, np.float64) + wqkv_b
    q, k, v = np.split(qkv, 3, axis=-1)
    th = lambda t: t.reshape(B, L, H, HEAD_DIM).transpose(0, 2, 1, 3)
    q, k, v = th(q), th(k), th(v)

    half = HEAD_DIM // 2
    inv_freq = 10000.0 ** (-np.arange(0, half) * 2.0 / HEAD_DIM)
    theta = np.arange(L)[:, None] * inv_freq[None, :]
    cos, sin = np.cos(theta), np.sin(theta)

    def rope(t):
        x1, x2 = t[..., :half], t[..., half:]
        return np.concatenate([x1 * cos - x2 * sin, x1 * sin + x2 * cos], axis=-1)

    q, k = rope(q), rope(k)
    s = np.einsum("bhqd,bhkd->bhqk", q / np.sqrt(HEAD_DIM), k) + np.asarray(mask, np.float64)
    s = s - s.max(axis=-1, keepdims=True)
    p = np.exp(s)
    p /= p.sum(axis=-1, keepdims=True)
    o = np.einsum("bhqk,bhkd->bhqd", p, v)
    o = o.transpose(0, 2, 1, 3).reshape(B, L, DIM)
    h = x + o @ np.asarray(out_w, np.float64) + out_b
    f = rms(h, ffn_scale)
    f = f @ np.asarray(lin1_w, np.float64) + lin1_b
    f = 0.5 * f * (1.0 + erf(f / np.sqrt(2.0)))
    mu = f.mean(axis=-1, keepdims=True)
    var = f.var(axis=-1, keepdims=True)
    f = (f - mu) / np.sqrt(var + LN_EPS) * ln_g + ln_b
    out = h + f @ np.asarray(lin2_w, np.float64) + lin2_b
    return out.astype(np.float32)


# ----------------------------------------------------------------------------
# device program
# ----------------------------------------------------------------------------

def _rms_rstd(nc, scratch, stats, t, eps):
    """1/sqrt(mean(t^2, free) + eps) for a [128, D] f32 tile, via ACT."""
    D = t.shape[1]
    sq = scratch.tile([128, D], BF16, tag="sq", name="sq")
    ssq = stats.tile([128, 1], F32, tag="ssq", name="ssq")
    nc.scalar.activation(sq[:], t[:], AF.Square, accum_out=ssq[:])
    std = stats.tile([128, 1], F32, tag="rmssd", name="rmssd")
    nc.scalar.activation(std[:], ssq[:], AF.Sqrt, bias=eps, scale=1.0 / D)
    rstd = stats.tile([128, 1], F32, tag="rmsrs", name="rmsrs")
    nc.vector.reciprocal(rstd[:], std[:])
    return rstd


def _register_const(nc, value, dtype=F32):
    t = nc.alloc_sbuf_tensor(f"const-{dtype.name}-{value}", [128, 1], dtype)
    nc.gpsimd.memset(t.ap(), value)
    nc.const_aps.aps[(dtype, value)] = t.ap()


def _build_program(actions, n_pmask):
    nc = bacc.Bacc("TRN2", target_bir_lowering=False, debug=False,
                   num_devices=NCORES)
    _register_const(nc, RMS_EPS)
    _register_const(nc, LN_EPS)
    nc.all_engine_barrier()

    x_in = nc.dram_tensor("x_own", [RC, DIM], F32, kind="ExternalInput")
    wqkv_in = nc.dram_tensor("wqkv_sl", [DIM, 3 * 128], BF16, kind="ExternalInput")
    outw_in = nc.dram_tensor("out_w", [DIM, DIM], BF16, kind="ExternalInput")
    l1w_in = nc.dram_tensor("lin1_w", [DIM, HID], BF16, kind="ExternalInput")
    l2w_in = nc.dram_tensor("lin2_w", [HID, DIM], BF16, kind="ExternalInput")
    cos_in = nc.dram_tensor("cosT", [128, L], F32, kind="ExternalInput")
    sin_in = nc.dram_tensor("sinT", [128, L], F32, kind="ExternalInput")
    pm_in = nc.dram_tensor("pmask", [n_pmask, 128, 512], BF16, kind="ExternalInput")
    eye_in = nc.dram_tensor("eye", [128, 128], BF16, kind="ExternalInput")
    y_out = nc.dram_tensor("y_own", [RC, DIM], F32, kind="ExternalOutput")

    with tile.TileContext(nc) as tc:
        _emit(nc, tc, x_in, wqkv_in, outw_in, l1w_in, l2w_in, cos_in, sin_in,
              pm_in, eye_in, y_out, actions, n_pmask)

    nc.compile()
    return nc


def _emit(nc, tc, x_in, wqkv_in, outw_in, l1w_in, l2w_in, cos_in, sin_in,
          pm_in, eye_in, y_out, actions, n_pmask):
    with (
        tc.tile_pool(name="dram", bufs=1, space="DRAM") as dram,
        tc.tile_pool(name="base", bufs=1) as base,
        tc.tile_pool(name="work", bufs=2) as work,
        tc.tile_pool(name="stats", bufs=4) as stats,
    ):
        eye = base.tile([128, 128], BF16)
        nc.sync.dma_start(eye[:], eye_in.ap())

        ag_in0 = dram.tile([DIM, RC // 2], BF16)
        ag_in1 = dram.tile([DIM, RC // 2], BF16)
        ag_out0 = dram.tile([NCORES * DIM, RC // 2], BF16, addr_space="Shared")
        ag_out1 = dram.tile([NCORES * DIM, RC // 2], BF16, addr_space="Shared")
        a2a_inA = dram.tile([NCORES * 65, RC], BF16)
        a2a_outA = dram.tile([NCORES * 65, RC], BF16)
        a2a_inB = dram.tile([NCORES * 65, RC], BF16)
        a2a_outB = dram.tile([NCORES * 65, RC], BF16)

        if True:
            # ---------------- phase 1: load x, rms-norm, transpose -> ag_in
            with (
                tc.tile_pool(name="ps1", bufs=2, space="PSUM") as ps1,
                tc.tile_pool(name="p1", bufs=3) as p1,
                tc.tile_pool(name="xp", bufs=2) as xp,
            ):
                for rt in range(8):
                    xt = xp.tile([128, DIM], F32, tag="x", name="xt")
                    nc.sync.dma_start(xt[:], x_in.ap()[128 * rt:128 * (rt + 1), :])
                    rstd = _rms_rstd(nc, p1, stats, xt, RMS_EPS)
                    xn = p1.tile([128, DIM], BF16, tag="xn", name="xn")
                    nc.scalar.activation(xn[:], xt[:], AF.Copy, bias=0.0,
                                         scale=rstd[:])
                    ev_all = p1.tile([128, 1024], BF16, tag="ev", name="ev")
                    for fc in range(8):
                        ps = ps1.tile([128, 128], BF16, tag="tr", name="tr")
                        nc.tensor.transpose(ps[:], xn[:, 128 * fc:128 * (fc + 1)], eye[:])
                        nc.vector.tensor_copy(ev_all[:, 128 * fc:128 * (fc + 1)],
                                              ps[:])
                    agd = ag_in0 if rt < 4 else ag_in1
                    lrt = rt % 4
                    nc.sync.dma_start(
                        agd[:, 128 * lrt:128 * (lrt + 1)]
                        .rearrange("(fc p) c -> p fc c", p=128),
                        ev_all[:].rearrange("p (fc c) -> p fc c", fc=8))
                    if rt == 3:
                        nc.gpsimd.collective_compute(
                            "AllGather", ALU.bypass,
                            replica_groups=[list(range(NCORES))],
                            ins=[ag_in0[:].opt()], outs=[ag_out0[:].opt()])
                    if rt == 7:
                        nc.gpsimd.collective_compute(
                            "AllGather", ALU.bypass,
                            replica_groups=[list(range(NCORES))],
                            ins=[ag_in1[:].opt()], outs=[ag_out1[:].opt()])

            with (
                tc.tile_pool(name="attn", bufs=1) as attn,
                tc.tile_pool(name="cst", bufs=1) as cst,
            ):
                cosT = cst.tile([128, L], F32)
                sinT = cst.tile([128, L], F32)
                nc.sync.dma_start(cosT[:], cos_in.ap())
                nc.sync.dma_start(sinT[:], sin_in.ap())
                pm_t = []
                for i in range(n_pmask):
                    t = cst.tile([128, 512], BF16, tag=f"pm{i}", name=f"pm{i}")
                    nc.sync.dma_start(t[:], pm_in.ap()[i, :, :])
                    pm_t.append(t)
                w_t = []
                for kc in range(8):
                    t = cst.tile([128, 3 * 128], BF16, tag=f"w{kc}", name=f"w{kc}")
                    nc.sync.dma_start(t[:], wqkv_in.ap()[128 * kc:128 * (kc + 1), :])
                    w_t.append(t)

                # QTA: head-A Q in rows 0-63, zeros in 64-127 (so S-matmuls
                # contract over all 128 partitions = full PE array = warm HAM).
                QTA = attn.tile([128, R], BF16)
                QTB = attn.tile([128, R], BF16)
                KT = attn.tile([128, R], BF16)
                VT = attn.tile([128, R], BF16)
                V_sb = attn.tile([128, 64 * VCOL], BF16)
                nc.vector.memset(V_sb[:], 1.0)
                nc.vector.memset(QTA[32:64, :], 0.0)
                nc.vector.memset(QTA[96:128, :], 0.0)
                nc.vector.memset(QTB[0:32, :], 0.0)
                nc.vector.memset(QTB[64:96, :], 0.0)

                # ------------ phase 3: QKV + RoPE
                with (
                    tc.tile_pool(name="ps3", bufs=2, space="PSUM") as ps3,
                    tc.tile_pool(name="p3", bufs=3) as p3,
                ):
                    for blk in [0, 2, 4, 6, 8, 10, 12, 14,
                                1, 3, 5, 7, 9, 11, 13, 15]:
                        cb, half = blk // 2, blk % 2
                        ag_src = ag_out0 if half == 0 else ag_out1
                        pos0 = 512 * (blk % 4)
                        g0 = 512 * blk
                        xnt_all = p3.tile([128, 4096], BF16, tag="xnt",
                                          name="xnt", bufs=2)
                        nc.sync.dma_start(
                            xnt_all[:].rearrange("p (kc c) -> p kc c", kc=8),
                            ag_src[DIM * cb:DIM * (cb + 1), :]
                            .rearrange("(kc p) c -> p kc c", p=128))
                        xnt = [xnt_all[:, 512 * kc:512 * (kc + 1)]
                               for kc in range(8)]
                        psX1 = ps3.tile([128, 512], F32, tag="q", name="psx1")
                        psX2 = ps3.tile([128, 512], F32, tag="k", name="psx2")
                        psV = ps3.tile([128, 512], F32, tag="v", name="psv")
                        for kc in range(8):
                            st, sp = kc == 0, kc == 7
                            nc.tensor.matmul(psX1[:], w_t[kc][:, 0:128], xnt[kc],
                                             start=st, stop=sp)
                            nc.tensor.matmul(psX2[:], w_t[kc][:, 128:256], xnt[kc],
                                             start=st, stop=sp)
                            nc.tensor.matmul(psV[:], w_t[kc][:, 256:384], xnt[kc],
                                             start=st, stop=sp)
                        # full-width rope: psX1 rows = [qx1A qx1B kx1A kx1B],
                        # psX2 rows = [qx2A qx2B kx2A kx2B]
                        cs = cosT[:, pos0:pos0 + 512]
                        sn = sinT[:, pos0:pos0 + 512]
                        t1 = work.tile([128, 512], F32, tag="r1", name="r1")
                        t2 = work.tile([128, 512], F32, tag="r2", name="r2")
                        o1 = p3.tile([128, 512], BF16, tag="o1", name="o1")
                        o2 = p3.tile([128, 512], BF16, tag="o2", name="o2")
                        nc.vector.tensor_mul(t1[:], psX1[:], cs)
                        nc.vector.tensor_mul(t2[:], psX2[:], sn)
                        nc.vector.tensor_sub(o1[:], t1[:], t2[:])
                        nc.vector.tensor_mul(t1[:], psX1[:], sn)
                        nc.vector.tensor_mul(t2[:], psX2[:], cs)
                        nc.vector.tensor_add(o2[:], t1[:], t2[:])
                        # reassemble: QTA rows 0-63 = head A, QTB rows 64-127
                        # = head B; KT = [headA d0-63, headB d0-63]
                        # KT rows = [kx1A kx1B kx2A kx2B]; QTA/QTB match with
                        # zeros in the other head's row slots.
                        for src_t, d0, dh, dst, dr in (
                            (o1, 0, 32, QTA, 0), (o2, 0, 32, QTA, 64),
                            (o1, 32, 32, QTB, 32), (o2, 32, 32, QTB, 96),
                            (o1, 64, 64, KT, 0), (o2, 64, 64, KT, 64),
                        ):
                            nc.scalar.dma_start(dst[dr:dr + dh, g0:g0 + 512],
                                                src_t[d0:d0 + dh, :])
                        nc.vector.tensor_copy(VT[:, g0:g0 + 512], psV[:])

                    # ------------ phase 4: V^T -> row-major V blocks (+ones)
                    for bt in range(64):
                        pst = ps3.tile([128, 128], BF16, tag="tr", name="tr")
                        nc.tensor.transpose(pst[:], VT[:, 128 * bt:128 * (bt + 1)],
                                            eye[:])
                        nc.vector.tensor_copy(V_sb[:, VCOL * bt:VCOL * bt + 64],
                                              pst[:, 0:64])
                        nc.vector.tensor_copy(V_sb[:, VCOL * bt + 65:VCOL * bt + 129],
                                              pst[:, 64:128])

                # ------------ phase 5: attention per (batch, head, qblock)
                with (
                    tc.tile_pool(name="ps5s", bufs=5, space="PSUM") as ps5s,
                    tc.tile_pool(name="ps5o", bufs=3, space="PSUM") as ps5o,
                    tc.tile_pool(name="p5", bufs=3) as p5,
                ):
                    LOOK = 3
                    for h in range(HPC):
                        hr = 64 * h
                        a2a_dst = a2a_inA if h == 0 else a2a_inB
                        for b in range(B):
                            for qb in range(4):
                                q0 = 2048 * b + 512 * qb
                                act = [(kt, actions[(kt, qb)]) for kt in range(16)
                                       if actions[(kt, qb)] != "skip"]
                                n = len(act)
                                psO = ps5o.tile([65, 512], F32, tag="o", name="pso")
                                pts = []
                                for i in range(n + LOOK):
                                    if i < n:
                                        kt, a = act[i]
                                        k0 = 2048 * b + 128 * kt
                                        psS = ps5s.tile([128, 512], F32, tag="s",
                                                        name="pss")
                                        QTh = QTA if h == 0 else QTB
                                        nc.tensor.matmul(
                                            psS[:], KT[:, k0:k0 + 128],
                                            QTh[:, q0:q0 + 512],
                                            start=True, stop=True)
                                        pt = p5.tile([128, 512], BF16, tag="pt",
                                                     name="pt", bufs=8)
                                        nc.scalar.activation(pt[:], psS[:], AF.Exp,
                                                             bias=0.0, scale=0.125)
                                        if a != "noop":
                                            nc.vector.tensor_mul(pt[:], pt[:],
                                                                 pm_t[a[1]][:])
                                        pts.append((kt, pt))
                                    j = i - LOOK
                                    if 0 <= j < n:
                                        kt, pt = pts[j]
                                        bt = 16 * b + kt
                                        nc.tensor.matmul(
                                            psO[:],
                                            V_sb[:, VCOL * bt + 65 * h:
                                                 VCOL * bt + 65 * h + 65],
                                            pt[:],
                                            start=(j == 0), stop=(j == n - 1))
                                ot65 = p5.tile([65, 512], BF16, tag="ot",
                                               name="ot65")
                                nc.vector.tensor_copy(ot65[:], psO[:, :])
                                j2 = (2048 * b + 512 * qb) // RC
                                loc0 = 2048 * b + 512 * qb - RC * j2
                                nc.sync.dma_start(
                                    a2a_dst[65 * j2:65 * j2 + 65,
                                            loc0:loc0 + 512],
                                    ot65[:])
                        nc.gpsimd.collective_compute(
                            "AllToAll", ALU.bypass,
                            replica_groups=[list(range(NCORES))],
                            ins=[(a2a_inA if h == 0 else a2a_inB)[:].opt()],
                            outs=[(a2a_outA if h == 0 else a2a_outB)[:].opt()])

            # ---------------- phase 7: out-proj + residual -> h
            with tc.tile_pool(name="hp", bufs=1) as hp:
                h_t = [hp.tile([128, DIM], F32, tag=f"h{rt}", name=f"h{rt}")
                       for rt in range(8)]
                with (
                    tc.tile_pool(name="ps7", bufs=1, space="PSUM") as ps7,
                    tc.tile_pool(name="p7", bufs=1) as p7,
                    tc.tile_pool(name="p7w", bufs=6) as p7w,
                ):
                    ow_t = []
                    for kc in range(8):
                        w = p7w.tile([128, 1024], BF16, tag="ow", name="ow",
                                     bufs=8)
                        nc.sync.dma_start(
                            w[:], outw_in.ap()[128 * kc:128 * (kc + 1), :])
                        ow_t.append(w)
                    x_t = []
                    for rt in range(8):
                        xt2 = p7.tile([128, DIM], F32, tag=f"x{rt}", name=f"x{rt}")
                        nc.sync.dma_start(xt2[:],
                                          x_in.ap()[128 * rt:128 * (rt + 1), :])
                        x_t.append(xt2)
                    oT = []
                    for kc in range(8):
                        t = p7.tile([128, RC], BF16, tag=f"ot{kc}", name=f"oT{kc}")
                        nc.sync.dma_start(t[0:64, :],
                                          a2a_outA[65 * kc:65 * kc + 64, :])
                        nc.sync.dma_start(t[64:128, :],
                                          a2a_outB[65 * kc:65 * kc + 64, :])
                        oT.append(t)
                    rs_sb = p7.tile([16, RC], BF16, tag="rs", name="rs_sb")
                    for kc in range(8):
                        nc.sync.dma_start(
                            rs_sb[2 * kc:2 * kc + 1, :],
                            a2a_outA[65 * kc + 64:65 * kc + 65, :])
                        nc.sync.dma_start(
                            rs_sb[2 * kc + 1:2 * kc + 2, :],
                            a2a_outB[65 * kc + 64:65 * kc + 65, :])
                    rs_rec = p7.tile([16, RC], F32, tag="rsr", name="rs_rec")
                    nc.vector.reciprocal(rs_rec[:], rs_sb[:])
                    rs_rb = p7.tile([16, RC], BF16, tag="rsb", name="rs_rb")
                    nc.vector.tensor_copy(rs_rb[:], rs_rec[:])
                    for kc in range(8):
                        bc = p7w.tile([128, RC], BF16, tag="bc", name="bc",
                                      bufs=3)
                        for hh in range(2):
                            r1 = p7w.tile([1, RC], BF16, tag="r1b", name="r1b",
                                          bufs=3)
                            nc.sync.dma_start(
                                r1[:], rs_rb[2 * kc + hh:2 * kc + hh + 1, :])
                            if hh == 0:
                                nc.gpsimd.partition_broadcast(bc[0:64, :], r1[:])
                            else:
                                tb = p7w.tile([64, RC], BF16, tag="tb",
                                              name="tb", bufs=3)
                                nc.gpsimd.partition_broadcast(tb[:], r1[:])
                                nc.sync.dma_start(bc[64:128, :], tb[:])
                        nc.vector.tensor_mul(oT[kc][:], oT[kc][:], bc[:])
                    for nb in range(2):
                        pss = [ps7.tile([128, 512], F32, tag=f"mm{rt % 4}",
                                        name="psmm", bufs=2) for rt in range(8)]
                        for kc in range(8):
                            w = ow_t[kc][:, 512 * nb:512 * (nb + 1)]
                            for rt in range(8):
                                nc.tensor.matmul(pss[rt][:],
                                                 oT[kc][:, 128 * rt:128 * (rt + 1)],
                                                 w, start=(kc == 0),
                                                 stop=(kc == 7))
                        for rt in range(8):
                            nc.vector.tensor_add(
                                h_t[rt][:, 512 * nb:512 * (nb + 1)], pss[rt][:],
                                x_t[rt][:, 512 * nb:512 * (nb + 1)])

                # ------------ phase 8: FFN (row-local), two halves of 512 rows
                with (
                    tc.tile_pool(name="ps8", bufs=1, space="PSUM") as ps8,
                    tc.tile_pool(name="ps8t", bufs=2, space="PSUM") as ps8t,
                    tc.tile_pool(name="p8", bufs=1) as p8,
                    tc.tile_pool(name="p8w", bufs=6) as p8w,
                    tc.tile_pool(name="p8s", bufs=3) as p8s,
                ):
                    for half in range(2):
                        # rms-norm h -> fn (bf16) -> transpose -> fnT
                        fnT = [p8.tile([128, 512], BF16, tag=f"fnT{fc}",
                                       name=f"fnT{fc}") for fc in range(8)]
                        for rt2 in range(4):
                            rt = 4 * half + rt2
                            rstd = _rms_rstd(nc, p8s, stats, h_t[rt], RMS_EPS)
                            fn = p8s.tile([128, DIM], BF16, tag="fn", name="fn")
                            nc.scalar.activation(fn[:], h_t[rt][:], AF.Copy,
                                                 bias=0.0, scale=rstd[:])
                            for fc in range(8):
                                ps = ps8t.tile([128, 128], BF16, tag="tr", name="tr")
                                nc.tensor.transpose(
                                    ps[:], fn[:, 128 * fc:128 * (fc + 1)], eye[:])
                                nc.vector.tensor_copy(
                                    fnT[fc][:, 128 * rt2:128 * (rt2 + 1)], ps[:])
                        # lin1 + GELU -> g [4 x 8 tiles of [128,512] bf16]
                        g_t = [[p8.tile([128, 512], BF16, tag=f"g{rt2}_{hb}",
                                        name=f"g{rt2}_{hb}")
                                for hb in range(8)] for rt2 in range(4)]
                        for hb in range(8):
                            pss = [ps8.tile([128, 512], F32, tag=f"mm{rt2}",
                                            name="psmm", bufs=1)
                                   for rt2 in range(4)]
                            wa = p8w.tile([128, 4096], BF16, tag="l1w",
                                          name="l1w", bufs=3)
                            nc.sync.dma_start(
                                wa[:].rearrange("p (fc c) -> p fc c", fc=8),
                                l1w_in.ap()[:, 512 * hb:512 * (hb + 1)]
                                .rearrange("(fc p) c -> p fc c", p=128))
                            for fc in range(8):
                                for rt2 in range(4):
                                    nc.tensor.matmul(
                                        pss[rt2][:],
                                        fnT[fc][:, 128 * rt2:128 * (rt2 + 1)],
                                        wa[:, 512 * fc:512 * (fc + 1)],
                                        start=(fc == 0), stop=(fc == 7))
                            for rt2 in range(4):
                                nc.scalar.activation(g_t[rt2][hb][:], pss[rt2][:],
                                                     AF.Gelu)
                        # LayerNorm stats over hid (4096) per row
                        ab = []
                        for rt2 in range(4):
                            st = stats.tile([128, 8, 6], F32, tag="lnst",
                                            name="lnst")
                            for hb in range(8):
                                nc.vector.bn_stats(st[:, hb, :], g_t[rt2][hb][:])
                            mv = stats.tile([128, 2], F32, tag="lnmv", name="lnmv")
                            nc.vector.bn_aggr(mv[:], st[:])
                            std = stats.tile([128, 1], F32, tag="lnsd", name="lnsd")
                            nc.scalar.activation(std[:], mv[:, 1:2], AF.Sqrt,
                                                 bias=LN_EPS, scale=1.0)
                            rstd = stats.tile([128, 1], F32, tag="lnrs",
                                              name="lnrs")
                            nc.vector.reciprocal(rstd[:], std[:])
                            nmr = stats.tile([128, 1], F32, tag="lnnm", name="lnnm")
                            nc.vector.tensor_scalar(nmr[:], rstd[:], mv[:, 0:1],
                                                    -1.0, ALU.mult, ALU.mult)
                            ab.append((rstd, nmr))
                        # normalize + transpose -> gnT [32 tiles of [128,512]]
                        gnT = [p8.tile([128, 512], BF16, tag=f"gnT{hc}",
                                       name=f"gnT{hc}") for hc in range(32)]
                        for rt2 in range(4):
                            rstd, nmr = ab[rt2]
                            for hb in range(8):
                                gn = p8s.tile([128, 512], BF16, tag="gn", name="gn")
                                nc.vector.tensor_scalar(gn[:], g_t[rt2][hb][:],
                                                        rstd[:], nmr[:],
                                                        ALU.mult, ALU.add)
                                for j in range(4):
                                    ps = ps8t.tile([128, 128], BF16, tag="tr",
                                                   name="tr")
                                    nc.tensor.transpose(
                                        ps[:], gn[:, 128 * j:128 * (j + 1)], eye[:])
                                    nc.vector.tensor_copy(
                                        gnT[4 * hb + j][:, 128 * rt2:128 * (rt2 + 1)],
                                        ps[:])
                        # lin2 + residual -> y
                        for nb in range(2):
                            pss = [ps8.tile([128, 512], F32, tag=f"mm{rt2}",
                                            name="psmm", bufs=1)
                                   for rt2 in range(4)]
                            for gg in range(4):
                                wa = p8w.tile([128, 4096], BF16, tag="l2w",
                                              name="l2w", bufs=3)
                                nc.sync.dma_start(
                                    wa[:].rearrange("p (j c) -> p j c", j=8),
                                    l2w_in.ap()[1024 * gg:1024 * (gg + 1),
                                                512 * nb:512 * (nb + 1)]
                                    .rearrange("(j p) c -> p j c", p=128))
                                for j in range(8):
                                    hc = 8 * gg + j
                                    for rt2 in range(4):
                                        nc.tensor.matmul(
                                            pss[rt2][:],
                                            gnT[hc][:, 128 * rt2:128 * (rt2 + 1)],
                                            wa[:, 512 * j:512 * (j + 1)],
                                            start=(hc == 0), stop=(hc == 31))
                            for rt2 in range(4):
                                rt = 4 * half + rt2
                                yt = p8s.tile([128, 512], F32, tag="yt", name="yt")
                                nc.vector.tensor_add(
                                    yt[:], pss[rt2][:],
                                    h_t[rt][:, 512 * nb:512 * (nb + 1)])
                                nc.sync.dma_start(
                                    y_out.ap()[128 * rt:128 * (rt + 1),
                                               512 * nb:512 * (nb + 1)],
                                    yt[:])


# ----------------------------------------------------------------------------
# entry point
# ----------------------------------------------------------------------------

def kernel(x, mask, attn_scale, wqkv_w, wqkv_b, out_w, out_b,
           ffn_scale, lin1_w, lin1_b, ln_g, ln_b, lin2_w, lin2_b):
    x = np.asarray(x, np.float32)
    mask = np.asarray(mask, np.float32)

    lin2_b_eff = (np.asarray(lin2_b, np.float32)
                  + np.asarray(ln_b, np.float32) @ np.asarray(lin2_w, np.float32))
    if np.any(wqkv_b) or np.any(out_b) or np.any(lin1_b) or np.any(lin2_b_eff):
        return _numpy_fallback(x, mask, attn_scale, wqkv_w, wqkv_b, out_w, out_b,
                               ffn_scale, lin1_w, lin1_b, ln_g, ln_b, lin2_w,
                               lin2_b)

    actions, pmask_np = _classify_mask(mask)
    for qb in range(4):
        if all(actions[(kt, qb)] == "skip" for kt in range(16)):
            return _numpy_fallback(x, mask, attn_scale, wqkv_w, wqkv_b, out_w,
                                   out_b, ffn_scale, lin1_w, lin1_b, ln_g, ln_b,
                                   lin2_w, lin2_b)

    mask_sig = tuple(sorted((k, str(v)) for k, v in actions.items()))
    key = (mask_sig, pmask_np.shape[0])
    if key not in _PROGRAM_CACHE:
        _PROGRAM_CACHE[key] = _build_program(actions, pmask_np.shape[0])
    nc = _PROGRAM_CACHE[key]

    asc = np.asarray(attn_scale, np.float32)
    wqkv_eff = asc[:, None] * np.asarray(wqkv_w, np.float32)
    wq, wk, wv = (wqkv_eff[:, :DIM], wqkv_eff[:, DIM:2 * DIM],
                  wqkv_eff[:, 2 * DIM:])
    out_w_bf = _bf16(out_w)
    l1_bf = _bf16(np.asarray(ffn_scale, np.float32)[:, None]
                  * np.asarray(lin1_w, np.float32))
    l2_bf = _bf16(np.asarray(lin2_w, np.float32)
                  * np.asarray(ln_g, np.float32)[:, None])
    cosT, sinT = _rope_tables()
    eye = np.eye(128, dtype=ml_dtypes.bfloat16)

    x2 = np.ascontiguousarray(x.reshape(R, DIM))
    in_maps = []
    for c in range(NCORES):
        hA, hB = 2 * c, 2 * c + 1
        qA, qB = wq[:, 64 * hA:64 * hA + 64], wq[:, 64 * hB:64 * hB + 64]
        kA, kB = wk[:, 64 * hA:64 * hA + 64], wk[:, 64 * hB:64 * hB + 64]
        # interleaved for full-width rope: [qx1A qx1B kx1A kx1B | x2... | vA vB]
        sl = np.concatenate([qA[:, :32], qB[:, :32], kA[:, :32], kB[:, :32],
                             qA[:, 32:], qB[:, 32:], kA[:, 32:], kB[:, 32:],
                             wv[:, 128 * c:128 * (c + 1)]], axis=1)
        in_maps.append(dict(
            x_own=np.ascontiguousarray(x2[RC * c:RC * (c + 1)]),
            wqkv_sl=_bf16(sl),
            out_w=out_w_bf,
            lin1_w=l1_bf,
            lin2_w=l2_bf,
            cosT=cosT,
            sinT=sinT,
            pmask=pmask_np,
            eye=eye,
        ))

    global _LAST_IN_MAPS
    _LAST_IN_MAPS = in_maps
    res = run_bass_kernel_spmd(nc, in_maps, core_ids=list(range(NCORES)))
    y = np.concatenate([res.results[c]["y_own"] for c in range(NCORES)], axis=0)
    return y.reshape(B, L, DIM).astype(np.float32)


# revision 19
# speedup vs baseline: 1.4459x; 1.0188x over previous
"""Trainium2 Bass kernel for nn_Block_30313879175568 (dense transformer block).

Sharding: head-parallel attention (2 heads/core on 8 cores) + row-parallel
FFN/out-proj (1024 rows/core). Collectives: AllGather of rms-normed
activations (bf16, transposed layout), AllToAll of attention outputs
(heads -> rows). All matmul operands bf16 (fp32 PSUM accumulation); vector
math fp32.

Self-contained: imports only installed packages (concourse et al.) + numpy.
"""

import numpy as np
import ml_dtypes

import concourse.bass as bass  # noqa: F401
import concourse.mybir as mybir
import concourse.tile as tile
from concourse import bacc
from concourse.bass_utils import run_bass_kernel_spmd

BF16 = mybir.dt.bfloat16
F32 = mybir.dt.float32
AF = mybir.ActivationFunctionType
ALU = mybir.AluOpType

B, L, DIM, H, HID = 4, 2048, 1024, 16, 4096
HEAD_DIM = 64
NCORES = 8
R = B * L              # 8192 global rows
RC = R // NCORES       # 1024 rows per core
HPC = H // NCORES      # 2 heads per core
RMS_EPS = 1e-6
LN_EPS = 1e-5
VCOL = 2 * (HEAD_DIM + 1)   # 130: V cols per (batch,ktile) block incl ones
SH = 2 * HEAD_DIM + 2        # 130: a2a shard rows: 2x64 o^T dims + 2 rowsum rows

_PROGRAM_CACHE = {}
_LAST_IN_MAPS = None


# ----------------------------------------------------------------------------
# host-side helpers
# ----------------------------------------------------------------------------

def _bf16(a):
    return np.asarray(a, dtype=np.float32).astype(ml_dtypes.bfloat16)


def _rope_tables():
    half = HEAD_DIM // 2
    inv_freq = 10000.0 ** (-np.arange(0, half, dtype=np.float32) * 2.0 / HEAD_DIM)
    pos = np.arange(L, dtype=np.float32)
    theta = pos[:, None] * inv_freq[None, :]          # [L, 32]
    cos = np.cos(theta).T.astype(np.float32)          # [32, L]
    sin = np.sin(theta).T.astype(np.float32)
    return (np.tile(cos, (4, 1)).copy(), np.tile(sin, (4, 1)).copy())  # [128, L]


def _classify_mask(mask):
    """Split mask^T [k, q] into (16 ktile x 4 qblock) blocks.

    Returns (actions, pmask_np): actions[(kt, qb)] is 'skip' | 'noop' |
    ('mul', idx); pmask_np is [NU, 128, 512] bf16 of exp(mask^T block).
    """
    maskT = np.asarray(mask, dtype=np.float32).T
    actions = {}
    uniq = {}
    tiles = []
    for qb in range(4):
        for kt in range(16):
            blk = maskT[128 * kt:128 * (kt + 1), 512 * qb:512 * (qb + 1)]
            if np.all(blk <= -30.0):
                actions[(kt, qb)] = "skip"
            elif np.all(blk == 0.0):
                actions[(kt, qb)] = "noop"
            else:
                pm = _bf16(np.tile(np.exp(blk.astype(np.float64)), (1, 2)))
                key = pm.tobytes()
                if key not in uniq:
                    uniq[key] = len(tiles)
                    tiles.append(pm)
                actions[(kt, qb)] = ("mul", uniq[key])
    if not tiles:
        tiles = [np.zeros((128, 1024), dtype=ml_dtypes.bfloat16)]
    pmask_np = np.stack(tiles, axis=0)
    return actions, pmask_np


def _numpy_fallback(x, mask, attn_scale, wqkv_w, wqkv_b, out_w, out_b,
                    ffn_scale, lin1_w, lin1_b, ln_g, ln_b, lin2_w, lin2_b):
    """Correct (slow) host fallback for configurations the device program
    doesn't support (nonzero biases / fully-masked rows)."""
    from scipy.special import erf

    def rms(t, scale):
        return t / np.sqrt(np.mean(t * t, axis=-1, keepdims=True) + RMS_EPS) * scale

    x = np.asarray(x, np.float64)
    xn = rms(x, attn_scale)
    qkv = xn @ np.asarray(wqkv_w, np.float64) + wqkv_b
    q, k, v = np.split(qkv, 3, axis=-1)
    th = lambda t: t.reshape(B, L, H, HEAD_DIM).transpose(0, 2, 1, 3)
    q, k, v = th(q), th(k), th(v)

    half = HEAD_DIM // 2
    inv_freq = 10000.0 ** (-np.arange(0, half) * 2.0 / HEAD_DIM)
    theta = np.arange(L)[:, None] * inv_freq[None, :]
    cos, sin = np.cos(theta), np.sin(theta)

    def rope(t):
        x1, x2 = t[..., :half], t[..., half:]
        return np.concatenate([x1 * cos - x2 * sin, x1 * sin + x2 * cos], axis=-1)

    q, k = rope(q), rope(k)
    s = np.einsum("bhqd,bhkd->bhqk", q / np.sqrt(HEAD_DIM), k) + np.asarray(mask, np.float64)
    s = s - s.max(axis=-1, keepdims=True)
    p = np.exp(s)
    p /= p.sum(axis=-1, keepdims=True)
    o = np.einsum("bhqk,bhkd->bhqd", p, v)
    o = o.transpose(0, 2, 1, 3).reshape(B, L, DIM)
    h = x + o @ np.asarray(out_w, np.float64) + out_b
    f = rms(h, ffn_scale)
    f = f @ np.asarray(lin1_w, np.float64) + lin1_b
    f = 0.5 * f * (1.0 + erf(f / np.sqrt(2.0)))
    mu = f.mean(axis=-1, keepdims=True)
    var = f.var(axis=-1, keepdims=True)
    f = (f - mu) / np.sqrt(var + LN_EPS) * ln_g + ln_b
    out = h + f @ np.asarray(lin2_w, np.float64) + lin2_b
    return out.astype(np.float32)


# ----------------------------------------------------------------------------
# device program
# ----------------------------------------------------------------------------

def _rms_rstd(nc, scratch, stats, t, eps):
    """1/sqrt(mean(t^2, free) + eps) for a [128, D] f32 tile, via ACT."""
    D = t.shape[1]
    sq = scratch.tile([128, D], BF16, tag="sq", name="sq")
    ssq = stats.tile([128, 1], F32, tag="ssq", name="ssq")
    nc.scalar.activation(sq[:], t[:], AF.Square, accum_out=ssq[:])
    std = stats.tile([128, 1], F32, tag="rmssd", name="rmssd")
    nc.scalar.activation(std[:], ssq[:], AF.Sqrt, bias=eps, scale=1.0 / D)
    rstd = stats.tile([128, 1], F32, tag="rmsrs", name="rmsrs")
    nc.vector.reciprocal(rstd[:], std[:])
    return rstd


def _register_const(nc, value, dtype=F32):
    t = nc.alloc_sbuf_tensor(f"const-{dtype.name}-{value}", [128, 1], dtype)
    nc.gpsimd.memset(t.ap(), value)
    nc.const_aps.aps[(dtype, value)] = t.ap()


def _build_program(actions, n_pmask):
    nc = bacc.Bacc("TRN2", target_bir_lowering=False, debug=False,
                   num_devices=NCORES)
    _register_const(nc, RMS_EPS)
    _register_const(nc, LN_EPS)
    nc.all_engine_barrier()

    x_in = nc.dram_tensor("x_own", [RC, DIM], F32, kind="ExternalInput")
    wqkv_in = nc.dram_tensor("wqkv_sl", [DIM, 3 * 128], BF16, kind="ExternalInput")
    outw_in = nc.dram_tensor("out_w", [DIM, DIM], BF16, kind="ExternalInput")
    l1w_in = nc.dram_tensor("lin1_w", [DIM, HID], BF16, kind="ExternalInput")
    l2w_in = nc.dram_tensor("lin2_w", [HID, DIM], BF16, kind="ExternalInput")
    cos_in = nc.dram_tensor("cosT", [128, L], F32, kind="ExternalInput")
    sin_in = nc.dram_tensor("sinT", [128, L], F32, kind="ExternalInput")
    pm_in = nc.dram_tensor("pmask", [n_pmask, 128, 1024], BF16, kind="ExternalInput")
    eye_in = nc.dram_tensor("eye", [128, 128], BF16, kind="ExternalInput")
    y_out = nc.dram_tensor("y_own", [RC, DIM], F32, kind="ExternalOutput")

    with tile.TileContext(nc) as tc:
        _emit(nc, tc, x_in, wqkv_in, outw_in, l1w_in, l2w_in, cos_in, sin_in,
              pm_in, eye_in, y_out, actions, n_pmask)

    nc.compile()
    return nc


def _emit(nc, tc, x_in, wqkv_in, outw_in, l1w_in, l2w_in, cos_in, sin_in,
          pm_in, eye_in, y_out, actions, n_pmask):
    with (
        tc.tile_pool(name="dram", bufs=1, space="DRAM") as dram,
        tc.tile_pool(name="base", bufs=1) as base,
        tc.tile_pool(name="work", bufs=2) as work,
        tc.tile_pool(name="stats", bufs=4) as stats,
    ):
        eye = base.tile([128, 128], BF16)
        nc.sync.dma_start(eye[:], eye_in.ap())

        ag_in0 = dram.tile([DIM, RC // 2], BF16)
        ag_in1 = dram.tile([DIM, RC // 2], BF16)
        ag_out0 = dram.tile([NCORES * DIM, RC // 2], BF16, addr_space="Shared")
        ag_out1 = dram.tile([NCORES * DIM, RC // 2], BF16, addr_space="Shared")
        a2a_inA = dram.tile([NCORES * 65, RC], BF16)
        a2a_outA = dram.tile([NCORES * 65, RC], BF16)
        a2a_inB = dram.tile([NCORES * 65, RC], BF16)
        a2a_outB = dram.tile([NCORES * 65, RC], BF16)

        if True:
            # ---------------- phase 1: load x, rms-norm, transpose -> ag_in
            with (
                tc.tile_pool(name="ps1", bufs=2, space="PSUM") as ps1,
                tc.tile_pool(name="p1", bufs=3) as p1,
                tc.tile_pool(name="xp", bufs=2) as xp,
            ):
                for rt in range(8):
                    xt = xp.tile([128, DIM], F32, tag="x", name="xt")
                    nc.sync.dma_start(xt[:], x_in.ap()[128 * rt:128 * (rt + 1), :])
                    rstd = _rms_rstd(nc, p1, stats, xt, RMS_EPS)
                    xn = p1.tile([128, DIM], BF16, tag="xn", name="xn")
                    nc.scalar.activation(xn[:], xt[:], AF.Copy, bias=0.0,
                                         scale=rstd[:])
                    ev_all = p1.tile([128, 1024], BF16, tag="ev", name="ev")
                    for fc in range(8):
                        ps = ps1.tile([128, 128], BF16, tag="tr", name="tr")
                        nc.tensor.transpose(ps[:], xn[:, 128 * fc:128 * (fc + 1)], eye[:])
                        nc.vector.tensor_copy(ev_all[:, 128 * fc:128 * (fc + 1)],
                                              ps[:])
                    agd = ag_in0 if rt < 4 else ag_in1
                    lrt = rt % 4
                    nc.sync.dma_start(
                        agd[:, 128 * lrt:128 * (lrt + 1)]
                        .rearrange("(fc p) c -> p fc c", p=128),
                        ev_all[:].rearrange("p (fc c) -> p fc c", fc=8))
                    if rt == 3:
                        nc.gpsimd.collective_compute(
                            "AllGather", ALU.bypass,
                            replica_groups=[list(range(NCORES))],
                            ins=[ag_in0[:].opt()], outs=[ag_out0[:].opt()])
                    if rt == 7:
                        nc.gpsimd.collective_compute(
                            "AllGather", ALU.bypass,
                            replica_groups=[list(range(NCORES))],
                            ins=[ag_in1[:].opt()], outs=[ag_out1[:].opt()])

            with (
                tc.tile_pool(name="attn", bufs=1) as attn,
                tc.tile_pool(name="cst", bufs=1) as cst,
            ):
                cosT = cst.tile([128, L], F32)
                sinT = cst.tile([128, L], F32)
                nc.sync.dma_start(cosT[:], cos_in.ap())
                nc.sync.dma_start(sinT[:], sin_in.ap())
                pm_t = []
                for i in range(n_pmask):
                    t = cst.tile([128, 1024], BF16, tag=f"pm{i}", name=f"pm{i}")
                    nc.sync.dma_start(t[:], pm_in.ap()[i, :, :])
                    pm_t.append(t)
                w_t = []
                for kc in range(8):
                    t = cst.tile([128, 3 * 128], BF16, tag=f"w{kc}", name=f"w{kc}")
                    nc.sync.dma_start(t[:], wqkv_in.ap()[128 * kc:128 * (kc + 1), :])
                    w_t.append(t)

                # QTA: head-A Q in rows 0-63, zeros in 64-127 (so S-matmuls
                # contract over all 128 partitions = full PE array = warm HAM).
                QTA = attn.tile([128, R], BF16)
                QTB = attn.tile([128, R], BF16)
                KT = attn.tile([128, R], BF16)
                VT = attn.tile([128, R], BF16)
                V_sb = attn.tile([128, 64 * VCOL], BF16)
                nc.vector.memset(V_sb[:], 1.0)
                nc.vector.memset(QTA[32:64, :], 0.0)
                nc.vector.memset(QTA[96:128, :], 0.0)
                nc.vector.memset(QTB[0:32, :], 0.0)
                nc.vector.memset(QTB[64:96, :], 0.0)

                # ------------ phase 3: QKV + RoPE
                with (
                    tc.tile_pool(name="ps3", bufs=2, space="PSUM") as ps3,
                    tc.tile_pool(name="p3", bufs=3) as p3,
                ):
                    for blk in [0, 2, 4, 6, 8, 10, 12, 14,
                                1, 3, 5, 7, 9, 11, 13, 15]:
                        cb, half = blk // 2, blk % 2
                        ag_src = ag_out0 if half == 0 else ag_out1
                        pos0 = 512 * (blk % 4)
                        g0 = 512 * blk
                        xnt_all = p3.tile([128, 4096], BF16, tag="xnt",
                                          name="xnt", bufs=2)
                        nc.sync.dma_start(
                            xnt_all[:].rearrange("p (kc c) -> p kc c", kc=8),
                            ag_src[DIM * cb:DIM * (cb + 1), :]
                            .rearrange("(kc p) c -> p kc c", p=128))
                        xnt = [xnt_all[:, 512 * kc:512 * (kc + 1)]
                               for kc in range(8)]
                        psX1 = ps3.tile([128, 512], F32, tag="q", name="psx1")
                        psX2 = ps3.tile([128, 512], F32, tag="k", name="psx2")
                        psV = ps3.tile([128, 512], F32, tag="v", name="psv")
                        for kc in range(8):
                            st, sp = kc == 0, kc == 7
                            nc.tensor.matmul(psX1[:], w_t[kc][:, 0:128], xnt[kc],
                                             start=st, stop=sp)
                            nc.tensor.matmul(psX2[:], w_t[kc][:, 128:256], xnt[kc],
                                             start=st, stop=sp)
                            nc.tensor.matmul(psV[:], w_t[kc][:, 256:384], xnt[kc],
                                             start=st, stop=sp)
                        # full-width rope: psX1 rows = [qx1A qx1B kx1A kx1B],
                        # psX2 rows = [qx2A qx2B kx2A kx2B]
                        cs = cosT[:, pos0:pos0 + 512]
                        sn = sinT[:, pos0:pos0 + 512]
                        t1 = work.tile([128, 512], F32, tag="r1", name="r1")
                        t2 = work.tile([128, 512], F32, tag="r2", name="r2")
                        o1 = p3.tile([128, 512], BF16, tag="o1", name="o1")
                        o2 = p3.tile([128, 512], BF16, tag="o2", name="o2")
                        nc.vector.tensor_mul(t1[:], psX1[:], cs)
                        nc.vector.tensor_mul(t2[:], psX2[:], sn)
                        nc.vector.tensor_sub(o1[:], t1[:], t2[:])
                        nc.vector.tensor_mul(t1[:], psX1[:], sn)
                        nc.vector.tensor_mul(t2[:], psX2[:], cs)
                        nc.vector.tensor_add(o2[:], t1[:], t2[:])
                        # reassemble: QTA rows 0-63 = head A, QTB rows 64-127
                        # = head B; KT = [headA d0-63, headB d0-63]
                        # KT rows = [kx1A kx1B kx2A kx2B]; QTA/QTB match with
                        # zeros in the other head's row slots.
                        for src_t, d0, dh, dst, dr in (
                            (o1, 0, 32, QTA, 0), (o2, 0, 32, QTA, 64),
                            (o1, 32, 32, QTB, 32), (o2, 32, 32, QTB, 96),
                            (o1, 64, 64, KT, 0), (o2, 64, 64, KT, 64),
                        ):
                            nc.scalar.dma_start(dst[dr:dr + dh, g0:g0 + 512],
                                                src_t[d0:d0 + dh, :])
                        nc.vector.tensor_copy(VT[:, g0:g0 + 512], psV[:])

                    # ------------ phase 4: V^T -> row-major V blocks (+ones)
                    for bt in range(64):
                        pst = ps3.tile([128, 128], BF16, tag="tr", name="tr")
                        nc.tensor.transpose(pst[:], VT[:, 128 * bt:128 * (bt + 1)],
                                            eye[:])
                        nc.vector.tensor_copy(V_sb[:, VCOL * bt:VCOL * bt + 64],
                                              pst[:, 0:64])
                        nc.vector.tensor_copy(V_sb[:, VCOL * bt + 65:VCOL * bt + 129],
                                              pst[:, 64:128])

                # ------------ phase 5: attention per (batch, head, qblock)
                with (
                    tc.tile_pool(name="ps5s", bufs=5, space="PSUM") as ps5s,
                    tc.tile_pool(name="ps5o", bufs=3, space="PSUM") as ps5o,
                    tc.tile_pool(name="p5", bufs=3) as p5,
                ):
                    LOOK = 3
                    for h in range(HPC):
                        hr = 64 * h
                        a2a_dst = a2a_inA if h == 0 else a2a_inB
                        QTh = QTA if h == 0 else QTB
                        for b2 in range(2):      # batch pair (b2, b2+2)
                            for qb in range(4):
                                act = [(kt, actions[(kt, qb)]) for kt in range(16)
                                       if actions[(kt, qb)] != "skip"]
                                n = len(act)
                                psOa = ps5o.tile([65, 512], F32, tag="oa",
                                                 name="psoa", bufs=2)
                                psOb = ps5o.tile([65, 512], F32, tag="ob",
                                                 name="psob", bufs=2)
                                pts = []
                                for i in range(n + LOOK):
                                    if i < n:
                                        kt, a = act[i]
                                        psS2 = ps5s.tile([128, 1024], F32,
                                                         tag="s", name="pss",
                                                         bufs=2)
                                        for bi, b in enumerate((b2, b2 + 2)):
                                            k0 = 2048 * b + 128 * kt
                                            q0 = 2048 * b + 512 * qb
                                            nc.tensor.matmul(
                                                psS2[:, 512 * bi:512 * (bi + 1)],
                                                KT[:, k0:k0 + 128],
                                                QTh[:, q0:q0 + 512],
                                                start=True, stop=True)
                                        pt2 = p5.tile([128, 1024], BF16,
                                                      tag="pt", name="pt",
                                                      bufs=6)
                                        nc.scalar.activation(pt2[:], psS2[:],
                                                             AF.Exp, bias=0.0,
                                                             scale=0.125)
                                        if a != "noop":
                                            nc.vector.tensor_mul(pt2[:], pt2[:],
                                                                 pm_t[a[1]][:])
                                        pts.append((kt, pt2))
                                    j = i - LOOK
                                    if 0 <= j < n:
                                        kt, pt2 = pts[j]
                                        for bi, b in enumerate((b2, b2 + 2)):
                                            bt = 16 * b + kt
                                            nc.tensor.matmul(
                                                (psOa if bi == 0 else psOb)[:],
                                                V_sb[:, VCOL * bt + 65 * h:
                                                     VCOL * bt + 65 * h + 65],
                                                pt2[:, 512 * bi:512 * (bi + 1)],
                                                start=(j == 0),
                                                stop=(j == n - 1))
                                for bi, b in enumerate((b2, b2 + 2)):
                                    ot65 = p5.tile([65, 512], BF16, tag="ot",
                                                   name="ot65")
                                    nc.vector.tensor_copy(
                                        ot65[:], (psOa if bi == 0 else psOb)[:, :])
                                    j2 = (2048 * b + 512 * qb) // RC
                                    loc0 = 2048 * b + 512 * qb - RC * j2
                                    nc.sync.dma_start(
                                        a2a_dst[65 * j2:65 * j2 + 65,
                                                loc0:loc0 + 512],
                                        ot65[:])
                        nc.gpsimd.collective_compute(
                            "AllToAll", ALU.bypass,
                            replica_groups=[list(range(NCORES))],
                            ins=[(a2a_inA if h == 0 else a2a_inB)[:].opt()],
                            outs=[(a2a_outA if h == 0 else a2a_outB)[:].opt()])

            # ---------------- phase 7: out-proj + residual -> h
            with tc.tile_pool(name="hp", bufs=1) as hp:
                h_t = [hp.tile([128, DIM], F32, tag=f"h{rt}", name=f"h{rt}")
                       for rt in range(8)]
                with (
                    tc.tile_pool(name="ps7", bufs=1, space="PSUM") as ps7,
                    tc.tile_pool(name="p7", bufs=1) as p7,
                    tc.tile_pool(name="p7w", bufs=6) as p7w,
                ):
                    ow_t = []
                    for kc in range(8):
                        w = p7w.tile([128, 1024], BF16, tag="ow", name="ow",
                                     bufs=8)
                        nc.sync.dma_start(
                            w[:], outw_in.ap()[128 * kc:128 * (kc + 1), :])
                        ow_t.append(w)
                    x_t = []
                    for rt in range(8):
                        xt2 = p7.tile([128, DIM], F32, tag=f"x{rt}", name=f"x{rt}")
                        nc.sync.dma_start(xt2[:],
                                          x_in.ap()[128 * rt:128 * (rt + 1), :])
                        x_t.append(xt2)
                    oT = []
                    for kc in range(8):
                        t = p7.tile([128, RC], BF16, tag=f"ot{kc}", name=f"oT{kc}")
                        nc.sync.dma_start(t[0:64, :],
                                          a2a_outA[65 * kc:65 * kc + 64, :])
                        nc.sync.dma_start(t[64:128, :],
                                          a2a_outB[65 * kc:65 * kc + 64, :])
                        oT.append(t)
                    rs_sb = p7.tile([16, RC], BF16, tag="rs", name="rs_sb")
                    for kc in range(8):
                        nc.sync.dma_start(
                            rs_sb[2 * kc:2 * kc + 1, :],
                            a2a_outA[65 * kc + 64:65 * kc + 65, :])
                        nc.sync.dma_start(
                            rs_sb[2 * kc + 1:2 * kc + 2, :],
                            a2a_outB[65 * kc + 64:65 * kc + 65, :])
                    rs_rec = p7.tile([16, RC], F32, tag="rsr", name="rs_rec")
                    nc.vector.reciprocal(rs_rec[:], rs_sb[:])
                    rs_rb = p7.tile([16, RC], BF16, tag="rsb", name="rs_rb")
                    nc.vector.tensor_copy(rs_rb[:], rs_rec[:])
                    for kc in range(8):
                        bc = p7w.tile([128, RC], BF16, tag="bc", name="bc",
                                      bufs=3)
                        for hh in range(2):
                            r1 = p7w.tile([1, RC], BF16, tag="r1b", name="r1b",
                                          bufs=3)
                            nc.sync.dma_start(
                                r1[:], rs_rb[2 * kc + hh:2 * kc + hh + 1, :])
                            if hh == 0:
                                nc.gpsimd.partition_broadcast(bc[0:64, :], r1[:])
                            else:
                                tb = p7w.tile([64, RC], BF16, tag="tb",
                                              name="tb", bufs=3)
                                nc.gpsimd.partition_broadcast(tb[:], r1[:])
                                nc.sync.dma_start(bc[64:128, :], tb[:])
                        nc.vector.tensor_mul(oT[kc][:], oT[kc][:], bc[:])
                    for nb in range(2):
                        pss = [ps7.tile([128, 512], F32, tag=f"mm{rt % 4}",
                                        name="psmm", bufs=2) for rt in range(8)]
                        for kc in range(8):
                            w = ow_t[kc][:, 512 * nb:512 * (nb + 1)]
                            for rt in range(8):
                                nc.tensor.matmul(pss[rt][:],
                                                 oT[kc][:, 128 * rt:128 * (rt + 1)],
                                                 w, start=(kc == 0),
                                                 stop=(kc == 7))
                        for rt in range(8):
                            nc.vector.tensor_add(
                                h_t[rt][:, 512 * nb:512 * (nb + 1)], pss[rt][:],
                                x_t[rt][:, 512 * nb:512 * (nb + 1)])

                # ------------ phase 8: FFN (row-local), two halves of 512 rows
                with (
                    tc.tile_pool(name="ps8", bufs=1, space="PSUM") as ps8,
                    tc.tile_pool(name="ps8t", bufs=2, space="PSUM") as ps8t,
                    tc.tile_pool(name="p8", bufs=1) as p8,
                    tc.tile_pool(name="p8w", bufs=6) as p8w,
                    tc.tile_pool(name="p8s", bufs=3) as p8s,
                ):
                    for half in range(2):
                        # rms-norm h -> fn (bf16) -> transpose -> fnT
                        fnT = [p8.tile([128, 512], BF16, tag=f"fnT{fc}",
                                       name=f"fnT{fc}") for fc in range(8)]
                        for rt2 in range(4):
                            rt = 4 * half + rt2
                            rstd = _rms_rstd(nc, p8s, stats, h_t[rt], RMS_EPS)
                            fn = p8s.tile([128, DIM], BF16, tag="fn", name="fn")
                            nc.scalar.activation(fn[:], h_t[rt][:], AF.Copy,
                                                 bias=0.0, scale=rstd[:])
                            for fc in range(8):
                                ps = ps8t.tile([128, 128], BF16, tag="tr", name="tr")
                                nc.tensor.transpose(
                                    ps[:], fn[:, 128 * fc:128 * (fc + 1)], eye[:])
                                nc.vector.tensor_copy(
                                    fnT[fc][:, 128 * rt2:128 * (rt2 + 1)], ps[:])
                        # lin1 + GELU -> g [4 x 8 tiles of [128,512] bf16]
                        g_t = [[p8.tile([128, 512], BF16, tag=f"g{rt2}_{hb}",
                                        name=f"g{rt2}_{hb}")
                                for hb in range(8)] for rt2 in range(4)]
                        for hb in range(8):
                            pss = [ps8.tile([128, 512], F32, tag=f"mm{rt2}",
                                            name="psmm", bufs=1)
                                   for rt2 in range(4)]
                            wa = p8w.tile([128, 4096], BF16, tag="l1w",
                                          name="l1w", bufs=3)
                            nc.sync.dma_start(
                                wa[:].rearrange("p (fc c) -> p fc c", fc=8),
                                l1w_in.ap()[:, 512 * hb:512 * (hb + 1)]
                                .rearrange("(fc p) c -> p fc c", p=128))
                            for fc in range(8):
                                for rt2 in range(4):
                                    nc.tensor.matmul(
                                        pss[rt2][:],
                                        fnT[fc][:, 128 * rt2:128 * (rt2 + 1)],
                                        wa[:, 512 * fc:512 * (fc + 1)],
                                        start=(fc == 0), stop=(fc == 7))
                            for rt2 in range(4):
                                nc.scalar.activation(g_t[rt2][hb][:], pss[rt2][:],
                                                     AF.Gelu)
                        # LayerNorm stats over hid (4096) per row
                        ab = []
                        for rt2 in range(4):
                            st = stats.tile([128, 8, 6], F32, tag="lnst",
                                            name="lnst")
                            for hb in range(8):
                                nc.vector.bn_stats(st[:, hb, :], g_t[rt2][hb][:])
                            mv = stats.tile([128, 2], F32, tag="lnmv", name="lnmv")
                            nc.vector.bn_aggr(mv[:], st[:])
                            std = stats.tile([128, 1], F32, tag="lnsd", name="lnsd")
                            nc.scalar.activation(std[:], mv[:, 1:2], AF.Sqrt,
                                                 bias=LN_EPS, scale=1.0)
                            rstd = stats.tile([128, 1], F32, tag="lnrs",
                                              name="lnrs")
                            nc.vector.reciprocal(rstd[:], std[:])
                            nmr = stats.tile([128, 1], F32, tag="lnnm", name="lnnm")
                            nc.vector.tensor_scalar(nmr[:], rstd[:], mv[:, 0:1],
                                                    -1.0, ALU.mult, ALU.mult)
                            ab.append((rstd, nmr))
                        # normalize + transpose -> gnT [32 tiles of [128,512]]
                        gnT = [p8.tile([128, 512], BF16, tag=f"gnT{hc}",
                                       name=f"gnT{hc}") for hc in range(32)]
                        for rt2 in range(4):
                            rstd, nmr = ab[rt2]
                            for hb in range(8):
                                gn = p8s.tile([128, 512], BF16, tag="gn", name="gn")
                                nc.vector.tensor_scalar(gn[:], g_t[rt2][hb][:],
                                                        rstd[:], nmr[:],
                                                        ALU.mult, ALU.add)
                                for j in range(4):
                                    ps = ps8t.tile([128, 128], BF16, tag="tr",
                                                   name="tr")
                                    nc.tensor.transpose(
                                        ps[:], gn[:, 128 * j:128 * (j + 1)], eye[:])
                                    nc.vector.tensor_copy(
                                        gnT[4 * hb + j][:, 128 * rt2:128 * (rt2 + 1)],
                                        ps[:])
                        # lin2 + residual -> y
                        for nb in range(2):
                            pss = [ps8.tile([128, 512], F32, tag=f"mm{rt2}",
                                            name="psmm", bufs=1)
                                   for rt2 in range(4)]
                            for gg in range(4):
                                wa = p8w.tile([128, 4096], BF16, tag="l2w",
                                              name="l2w", bufs=3)
                                nc.sync.dma_start(
                                    wa[:].rearrange("p (j c) -> p j c", j=8),
                                    l2w_in.ap()[1024 * gg:1024 * (gg + 1),
                                                512 * nb:512 * (nb + 1)]
                                    .rearrange("(j p) c -> p j c", p=128))
                                for j in range(8):
                                    hc = 8 * gg + j
                                    for rt2 in range(4):
                                        nc.tensor.matmul(
                                            pss[rt2][:],
                                            gnT[hc][:, 128 * rt2:128 * (rt2 + 1)],
                                            wa[:, 512 * j:512 * (j + 1)],
                                            start=(hc == 0), stop=(hc == 31))
                            for rt2 in range(4):
                                rt = 4 * half + rt2
                                yt = p8s.tile([128, 512], F32, tag="yt", name="yt")
                                nc.vector.tensor_add(
                                    yt[:], pss[rt2][:],
                                    h_t[rt][:, 512 * nb:512 * (nb + 1)])
                                nc.sync.dma_start(
                                    y_out.ap()[128 * rt:128 * (rt + 1),
                                               512 * nb:512 * (nb + 1)],
                                    yt[:])


# ----------------------------------------------------------------------------
# entry point
# ----------------------------------------------------------------------------

def kernel(x, mask, attn_scale, wqkv_w, wqkv_b, out_w, out_b,
           ffn_scale, lin1_w, lin1_b, ln_g, ln_b, lin2_w, lin2_b):
    x = np.asarray(x, np.float32)
    mask = np.asarray(mask, np.float32)

    lin2_b_eff = (np.asarray(lin2_b, np.float32)
                  + np.asarray(ln_b, np.float32) @ np.asarray(lin2_w, np.float32))
    if np.any(wqkv_b) or np.any(out_b) or np.any(lin1_b) or np.any(lin2_b_eff):
        return _numpy_fallback(x, mask, attn_scale, wqkv_w, wqkv_b, out_w, out_b,
                               ffn_scale, lin1_w, lin1_b, ln_g, ln_b, lin2_w,
                               lin2_b)

    actions, pmask_np = _classify_mask(mask)
    for qb in range(4):
        if all(actions[(kt, qb)] == "skip" for kt in range(16)):
            return _numpy_fallback(x, mask, attn_scale, wqkv_w, wqkv_b, out_w,
                                   out_b, ffn_scale, lin1_w, lin1_b, ln_g, ln_b,
                                   lin2_w, lin2_b)

    mask_sig = tuple(sorted((k, str(v)) for k, v in actions.items()))
    key = (mask_sig, pmask_np.shape[0])
    if key not in _PROGRAM_CACHE:
        _PROGRAM_CACHE[key] = _build_program(actions, pmask_np.shape[0])
    nc = _PROGRAM_CACHE[key]

    asc = np.asarray(attn_scale, np.float32)
    wqkv_eff = asc[:, None] * np.asarray(wqkv_w, np.float32)
    wq, wk, wv = (wqkv_eff[:, :DIM], wqkv_eff[:, DIM:2 * DIM],
                  wqkv_eff[:, 2 * DIM:])
    out_w_bf = _bf16(out_w)
    l1_bf = _bf16(np.asarray(ffn_scale, np.float32)[:, None]
                  * np.asarray(lin1_w, np.float32))
    l2_bf = _bf16(np.asarray(lin2_w, np.float32)
                  * np.asarray(ln_g, np.float32)[:, None])
    cosT, sinT = _rope_tables()
    eye = np.eye(128, dtype=ml_dtypes.bfloat16)

    x2 = np.ascontiguousarray(x.reshape(R, DIM))
    in_maps = []
    for c in range(NCORES):
        hA, hB = 2 * c, 2 * c + 1
        qA, qB = wq[:, 64 * hA:64 * hA + 64], wq[:, 64 * hB:64 * hB + 64]
        kA, kB = wk[:, 64 * hA:64 * hA + 64], wk[:, 64 * hB:64 * hB + 64]
        # interleaved for full-width rope: [qx1A qx1B kx1A kx1B | x2... | vA vB]
        sl = np.concatenate([qA[:, :32], qB[:, :32], kA[:, :32], kB[:, :32],
                             qA[:, 32:], qB[:, 32:], kA[:, 32:], kB[:, 32:],
                             wv[:, 128 * c:128 * (c + 1)]], axis=1)
        in_maps.append(dict(
            x_own=np.ascontiguousarray(x2[RC * c:RC * (c + 1)]),
            wqkv_sl=_bf16(sl),
            out_w=out_w_bf,
            lin1_w=l1_bf,
            lin2_w=l2_bf,
            cosT=cosT,
            sinT=sinT,
            pmask=pmask_np,
            eye=eye,
        ))

    global _LAST_IN_MAPS
    _LAST_IN_MAPS = in_maps
    res = run_bass_kernel_spmd(nc, in_maps, core_ids=list(range(NCORES)))
    y = np.concatenate([res.results[c]["y_own"] for c in range(NCORES)], axis=0)
    return y.reshape(B, L, DIM).astype(np.float32)
